# revision 18
# baseline (speedup 1.0000x reference)
"""COTREC GNN message-passing kernel for 8 Trainium2 NeuronCores.

Strategy (sharding_hint: row-shard sparse mm + all-gather; data-parallel
sessions):
  - Nodes are degree-sorted and striped across the 8 cores so every core gets
    an identical per-row-tile degree profile (one SPMD program).
  - The item table is stored fp16, unpadded (E=100 -> 200B rows), laid out in
    the chunk-major AllGather order.
  - Layer 1 (h1 = A @ emb): row tiles are batched into groups with a uniform
    neighbor-slot count k; ONE indirect DMA per group gathers all 128*k*Tg
    neighbor rows (SWDGE fixed cost ~1us amortizes over the whole group).
    DVE does one broadcast multiply by edge values and a log2(k) in-place
    pairwise-add tree (fp16 fast modes). h1 chunks are AllGathered (fp16),
    overlapped with compute.
  - Layer 2 (h2 = A @ h1) is computed only for nodes referenced by sessions
    (~23% of nodes); (emb + h1 + h2)/3 rows are packed into a compact fp16
    table and AllGathered.
  - Stage 2 (session attention pooling) is data-parallel over sessions (64
    per core) in feature-major layout: PE matmuls + ACT tanh/sigmoid + DVE.
Host side does only index preprocessing / sharding / packing (numpy).
"""
import sys

sys.path.insert(0, "/opt/trn_rl_repo")

import numpy as np

NCORES = 8
E = 100  # feature dim, rows stored unpadded fp16 (200B)


def _greedy_groups(ks, lo, hi, max_cols=240, waste=0.06):
    """Group consecutive tiles [lo,hi) (ks descending-ish) with uniform k.

    Returns list of (t0, Tg, kg). Waste = kg*Tg - sum(k) bounded."""
    out = []
    i = lo
    while i < hi:
        kmax = int(ks[i])
        sum_k = int(ks[i])
        j = i + 1
        while j < hi:
            km = max(kmax, int(ks[j]))
            n = j - i + 1
            s = sum_k + int(ks[j])
            if km * n > max_cols:
                break
            if km * n - s > max(waste * s, 4):
                break
            kmax, sum_k = km, s
            j += 1
        out.append((i, j - i, kmax))
        i = j
    return out


# --------------------------------------------------------------------------
# host preprocessing
# --------------------------------------------------------------------------
def _preprocess(inputs):
    rows = np.asarray(inputs["adj_rows"]).astype(np.int64).ravel()
    cols = np.asarray(inputs["adj_cols"]).astype(np.int64).ravel()
    vals = np.asarray(inputs["adj_vals"]).astype(np.float32).ravel()
    emb = np.asarray(inputs["embedding"]).astype(np.float32)
    sess = np.asarray(inputs["reversed_sess_item"]).astype(np.int64)
    mask = np.asarray(inputs["mask"]).astype(np.float32)
    slen = np.asarray(inputs["session_len"]).astype(np.float32)
    posemb = np.asarray(inputs["pos_embedding"]).astype(np.float32)

    N, E_ = emb.shape
    assert E_ == E
    B, L = sess.shape
    assert B % NCORES == 0
    B_LOC = B // NCORES
    RLOC = -(-N // NCORES)
    T1 = -(-RLOC // 128)
    RPAD = T1 * 128
    if RLOC == RPAD:
        T1 += 1
        RPAD += 128

    # AllGather chunking: h1_full is chunk-major: [chunk][rank][rows-in-chunk]
    nch = next(d for d in (7, 8, 6, 5, 4, 3, 2, 1) if T1 % d == 0)
    CHT = T1 // nch  # tiles per chunk
    CH = CHT * 128  # rows per chunk per core

    deg = np.bincount(rows, minlength=N).astype(np.int64)
    order = np.argsort(-deg, kind="stable")
    pos_of = np.empty(N, np.int64)
    pos_of[order] = np.arange(N)
    deg_sorted = deg[order]
    # table position of node n in the chunk-major AG layout
    _c = pos_of % NCORES
    _s = pos_of // NCORES
    tab_of = (_s // CH) * (NCORES * CH) + _c * CH + (_s % CH)

    # per-tile max degree (stripe of 128*NCORES sorted nodes)
    stripe_starts = np.minimum(np.arange(T1) * (128 * NCORES), N - 1)
    k1 = np.maximum(1, deg_sorted[stripe_starts]).astype(np.int64)

    # groups within AG chunks
    groups1 = []
    for c in range(nch):
        groups1 += _greedy_groups(k1, c * CHT, (c + 1) * CHT)
    g_of_tile1 = np.empty(T1, np.int64)
    moff1 = np.empty(len(groups1), np.int64)
    off = 0
    for gi, (t0, Tg, kg) in enumerate(groups1):
        g_of_tile1[t0 : t0 + Tg] = gi
        moff1[gi] = off
        off += 128 * kg * Tg
    S1G = off

    # edge -> (core, tile, partition, j)
    p_e = pos_of[rows]
    c_e = (p_e % NCORES).astype(np.int64)
    loc_e = p_e // NCORES
    ordE = np.argsort(p_e, kind="stable")
    pe_s = p_e[ordE]
    j_s = np.arange(len(rows)) - np.searchsorted(pe_s, pe_s, side="left")
    j_e = np.empty(len(rows), np.int64)
    j_e[ordE] = j_s
    t_e = loc_e // 128
    part_e = loc_e % 128
    # meta layout per group: [p][j*Tg + (t - t0)] (k-major columns)
    g_e = g_of_tile1[t_e]
    t0_e = np.array([g[0] for g in groups1])[g_e]
    Tg_e = np.array([g[1] for g in groups1])[g_e]
    kg_e = np.array([g[2] for g in groups1])[g_e]
    mpos = moff1[g_e] + part_e * (kg_e * Tg_e) + j_e * Tg_e + (t_e - t0_e)
    col_tab = tab_of[cols].astype(np.int32)
    pk1 = np.zeros((NCORES, S1G), np.int32)
    pv1 = np.zeros((NCORES, S1G), np.float16)
    pk1[c_e, mpos] = col_tab
    pv1[c_e, mpos] = vals.astype(np.float16)

    # positioned fp16 table (zero rows for pad positions)
    emb_pos = np.zeros((NCORES * RPAD, E), np.float16)
    emb_pos[tab_of[np.arange(N)]] = emb.astype(np.float16)

    # ---------------- layer 2 (needed nodes only) ----------------
    s_nodes = np.unique(sess[sess > 0]).astype(np.int64) - 1
    s_sorted = s_nodes[np.argsort(-deg[s_nodes], kind="stable")]
    NS = len(s_sorted)
    R2 = -(-NS // NCORES)
    T2 = -(-R2 // 128)
    R2P = T2 * 128
    used0 = -(-NS // NCORES)  # occupied local slots on core 0
    if used0 >= R2P:  # need a free (all-zero) pad slot for idx==0 sessions
        T2 += 1
        R2P += 128
    spos = np.full(N, -1, np.int64)
    spos[s_sorted] = np.arange(NS)

    cs2 = np.minimum(np.arange(T2) * (128 * NCORES), NS - 1)
    k2 = (np.maximum(1, deg[s_sorted[cs2]]) + 1).astype(np.int64)  # +1 self

    groups2 = _greedy_groups(k2, 0, T2)
    g_of_tile2 = np.empty(T2, np.int64)
    moff2 = np.empty(len(groups2), np.int64)
    off = 0
    for gi, (t0, Tg, kg) in enumerate(groups2):
        g_of_tile2[t0 : t0 + Tg] = gi
        moff2[gi] = off
        off += 128 * kg * Tg
    S2G = off

    pk2 = np.zeros((NCORES, S2G), np.int32)
    pv2 = np.zeros((NCORES, S2G), np.float16)
    # self slot j=0: + h1[i]/3
    q = np.arange(NS)
    qc = q % NCORES
    qs = q // NCORES
    qt = qs // 128
    qp = qs % 128
    g_q = g_of_tile2[qt]
    t0_q = np.array([g[0] for g in groups2])[g_q]
    Tg_q = np.array([g[1] for g in groups2])[g_q]
    kg_q = np.array([g[2] for g in groups2])[g_q]
    mq = moff2[g_q] + qp * (kg_q * Tg_q) + 0 * Tg_q + (qt - t0_q)
    pk2[qc, mq] = tab_of[s_sorted].astype(np.int32)
    pv2[qc, mq] = np.float16(1.0 / 3.0)
    # edges with row in S, slots j>=1
    maskE = spos[rows] >= 0
    q_e2 = spos[rows[maskE]]
    ordE2 = np.argsort(q_e2, kind="stable")
    qe_s = q_e2[ordE2]
    j2_s = np.arange(len(qe_s)) - np.searchsorted(qe_s, qe_s, side="left")
    j2 = np.empty(len(qe_s), np.int64)
    j2[ordE2] = j2_s
    c2 = q_e2 % NCORES
    s2 = q_e2 // NCORES
    t2_ = s2 // 128
    p2 = s2 % 128
    g_2 = g_of_tile2[t2_]
    t0_2 = np.array([g[0] for g in groups2])[g_2]
    Tg_2 = np.array([g[1] for g in groups2])[g_2]
    kg_2 = np.array([g[2] for g in groups2])[g_2]
    mpos2 = moff2[g_2] + p2 * (kg_2 * Tg_2) + (1 + j2) * Tg_2 + (t2_ - t0_2)
    pk2[c2, mpos2] = col_tab[maskE]
    pv2[c2, mpos2] = (vals[maskE] / 3.0).astype(np.float16)

    # host-gathered emb/3 rows, pre-swizzled per group: flat [p][t][e]
    h0rows = np.zeros((NCORES, R2P, E), np.float16)
    h0rows[qc, qs] = (emb[s_sorted] / 3.0).astype(np.float16)
    h0sw = np.zeros((NCORES, T2 * 128 * E), np.float16)
    hoff = {}
    off = 0
    for gi, (t0, Tg, kg) in enumerate(groups2):
        hoff[gi] = off
        blk = h0rows[:, t0 * 128 : (t0 + Tg) * 128, :]  # [NC, Tg*128, E]
        blk = blk.reshape(NCORES, Tg, 128, E).transpose(0, 2, 1, 3)  # [NC,p,t,e]
        h0sw[:, off : off + 128 * Tg * E] = blk.reshape(NCORES, -1)
        off += 128 * Tg * E
    h0sw = h0sw[:, :off]
    H0SZ = off

    # ---------------- session routing ----------------
    pad_crow = 0 * R2P + used0  # all-zero row in compact table (core0 pad)
    crow = np.full((B, L), pad_crow, np.int64)
    nz = sess > 0
    qv = spos[sess[nz] - 1]
    assert (qv >= 0).all()
    crow[nz] = (qv % NCORES) * R2P + qv // NCORES
    SESS = B_LOC * L
    SJ = -(-SESS // 128)
    assert SJ * 128 == SESS, "SESS must be a multiple of 128"
    sidx = np.full((NCORES, 128, SJ), pad_crow, np.int32)
    for c in range(NCORES):
        flat = crow[c * B_LOC : (c + 1) * B_LOC].ravel()  # r = b_loc*L + l
        rr = np.arange(SESS)
        sidx[c, rr % 128, rr // 128] = flat

    mask_c = mask.reshape(NCORES, 1, SESS).astype(np.float32)
    slen_c = slen.reshape(NCORES, 1, B_LOC).astype(np.float32)

    cfg = dict(
        N=N, B=B, L=L, B_LOC=B_LOC, RLOC=RLOC, RPAD=RPAD, T1=T1,
        T2=T2, R2P=R2P, S1G=S1G, S2G=S2G, H0SZ=H0SZ, SJ=SJ, SESS=SESS,
        NCH=nch, CHT=CHT,
        groups1=groups1, groups2=groups2,
        moff1=[int(x) for x in moff1], moff2=[int(x) for x in moff2],
        hoff=hoff,
    )
    per_core = []
    for c in range(NCORES):
        per_core.append(
            dict(
                emb_pos=emb_pos,
                pk1=pk1[c], pv1=pv1[c],
                pk2=pk2[c], pv2=pv2[c],
                h0sw=h0sw[c],
                sidx=sidx[c],
                pos50=np.ascontiguousarray(posemb[:L]),
                w1t=np.ascontiguousarray(np.asarray(inputs["w_1"])[:E].astype(np.float32)),
                w1b=np.ascontiguousarray(np.asarray(inputs["w_1"])[E:].astype(np.float32)),
                g1w=np.asarray(inputs["glu1_w"]).astype(np.float32),
                g1b=np.asarray(inputs["glu1_b"]).astype(np.float32).reshape(E, 1),
                g2w=np.asarray(inputs["glu2_w"]).astype(np.float32),
                w2=np.asarray(inputs["w_2"]).astype(np.float32),
                mask_c=mask_c[c],
                slen_c=slen_c[c],
            )
        )
    return cfg, per_core


# --------------------------------------------------------------------------
# device program
# --------------------------------------------------------------------------
def _build_program(cfg, stage="full", debug_taps=False):
    import concourse.bass as bass
    import concourse.bacc as bacc
    import concourse.mybir as mybir
    import concourse.tile as tile
    from concourse.masks import make_identity

    dt = mybir.dt
    f32, f16, i32 = dt.float32, dt.float16, dt.int32
    Alu = mybir.AluOpType
    Act = mybir.ActivationFunctionType
    X = mybir.AxisListType.X

    L = cfg["L"]
    B_LOC = cfg["B_LOC"]
    RPAD = cfg["RPAD"]
    T2 = cfg["T2"]
    R2P = cfg["R2P"]
    S1G = cfg["S1G"]
    S2G = cfg["S2G"]
    H0SZ = cfg["H0SZ"]
    SJ = cfg["SJ"]
    SESS = cfg["SESS"]
    groups1 = cfg["groups1"]
    groups2 = cfg["groups2"]
    moff1 = cfg["moff1"]
    moff2 = cfg["moff2"]
    hoff = cfg["hoff"]
    NCH = cfg["NCH"]
    CHT = cfg["CHT"]
    CH = CHT * 128
    rg = [list(range(NCORES))]

    nc = bacc.Bacc(
        "TRN2", target_bir_lowering=False, debug=False, num_devices=NCORES
    )
    emb_pos = nc.dram_tensor("emb_pos", [NCORES * RPAD, E], f16, kind="ExternalInput").ap()
    pk1_d = nc.dram_tensor("pk1", [S1G], i32, kind="ExternalInput").ap()
    pv1_d = nc.dram_tensor("pv1", [S1G], f16, kind="ExternalInput").ap()
    pk2_d = nc.dram_tensor("pk2", [S2G], i32, kind="ExternalInput").ap()
    pv2_d = nc.dram_tensor("pv2", [S2G], f16, kind="ExternalInput").ap()
    h0sw_d = nc.dram_tensor("h0sw", [H0SZ], f16, kind="ExternalInput").ap()
    sidx_d = nc.dram_tensor("sidx", [128, SJ], i32, kind="ExternalInput").ap()
    pos_d = nc.dram_tensor("pos50", [L, E], f32, kind="ExternalInput").ap()
    w1t_d = nc.dram_tensor("w1t", [E, E], f32, kind="ExternalInput").ap()
    w1b_d = nc.dram_tensor("w1b", [E, E], f32, kind="ExternalInput").ap()
    g1w_d = nc.dram_tensor("g1w", [E, E], f32, kind="ExternalInput").ap()
    g1b_d = nc.dram_tensor("g1b", [E, 1], f32, kind="ExternalInput").ap()
    g2w_d = nc.dram_tensor("g2w", [E, E], f32, kind="ExternalInput").ap()
    w2_d = nc.dram_tensor("w2", [E, 1], f32, kind="ExternalInput").ap()
    mask_d = nc.dram_tensor("mask_c", [1, SESS], f32, kind="ExternalInput").ap()
    slen_d = nc.dram_tensor("slen_c", [1, B_LOC], f32, kind="ExternalInput").ap()
    out_d = nc.dram_tensor("out", [B_LOC, E], f32, kind="ExternalOutput").ap()

    if debug_taps:
        g1_0 = cfg["groups1"][0]
        dbg_g = nc.dram_tensor(
            "dbg_g", [128, g1_0[1] * g1_0[2] * E], f16, kind="ExternalOutput"
        ).ap()
        dbg_h1 = nc.dram_tensor("dbg_h1", [RPAD, E], f16, kind="ExternalOutput").ap()
        dbg_cmp = nc.dram_tensor("dbg_cmp", [R2P, E], f16, kind="ExternalOutput").ap()
    h1_blk = nc.dram_tensor("h1_blk", [RPAD, E], f16, kind="Internal").ap()
    h1_full = nc.dram_tensor(
        "h1_full", [NCORES * RPAD, E], f16, kind="Internal", addr_space="Shared"
    ).ap()
    cmp_blk = nc.dram_tensor("cmp_blk", [R2P, E], f16, kind="Internal").ap()
    cmp_full = nc.dram_tensor(
        "cmp_full", [NCORES * R2P, E], f16, kind="Internal", addr_space="Shared"
    ).ap()

    from contextlib import ExitStack

    with tile.TileContext(nc) as tc, ExitStack() as ctx:
        res = ctx.enter_context(tc.tile_pool(name="res", bufs=1))
        mpool = ctx.enter_context(tc.tile_pool(name="meta", bufs=2))
        gpool = ctx.enter_context(tc.tile_pool(name="g", bufs=2))
        hpool = ctx.enter_context(tc.tile_pool(name="h0", bufs=2))

        def sparse_layer(groups, moff, pk_d, pv_d, src_tab, dst_blk, lname,
                         with_h0=False, post_group=None):
            pending = []  # delayed actions (AG triggers), flushed mid-gather
            for gi, (t0, Tg, kg) in enumerate(groups):
                Kg = kg * Tg
                TE = Tg * E
                mi = mpool.tile([128, Kg], i32, tag=f"{lname}i", name=f"{lname}i_{gi}")
                mv = mpool.tile([128, Kg], f16, tag=f"{lname}v", name=f"{lname}v_{gi}")
                nc.sync.dma_start(
                    out=mi[:],
                    in_=pk_d[moff[gi] : moff[gi] + 128 * Kg].rearrange(
                        "(p c) -> p c", c=Kg
                    ),
                )
                nc.sync.dma_start(
                    out=mv[:],
                    in_=pv_d[moff[gi] : moff[gi] + 128 * Kg].rearrange(
                        "(p c) -> p c", c=Kg
                    ),
                )
                g = gpool.tile([128, Kg * E], f16, tag="g")
                # one indirect DMA per slot column (the only offset/run pairing
                # the SWDGE ucode supports: one offset per partition)
                flush_at = Kg // 2
                for c in range(Kg):
                    if c == flush_at:
                        for fn in pending:
                            fn()
                        pending = []
                    nc.gpsimd.indirect_dma_start(
                        out=g[:, c * E : (c + 1) * E],
                        out_offset=None,
                        in_=src_tab,
                        in_offset=bass.IndirectOffsetOnAxis(
                            ap=mi[:, c : c + 1], axis=0
                        ),
                    )
                if debug_taps and lname == "m1" and gi == 0:
                    nc.sync.dma_start(out=dbg_g, in_=g[:])
                # multiply every gathered row by its edge value (k-major
                # layout: column block c = j*Tg + t, value per (j, t))
                g4 = g[:].rearrange("p (k t e) -> p k t e", t=Tg, e=E)
                mv4 = (
                    mv[:]
                    .rearrange("p (k t) -> p k t", t=Tg)
                    .unsqueeze(3)
                    .broadcast_to([128, kg, Tg, E])
                )
                nc.vector.scalar_tensor_tensor(
                    out=g4, in0=g4, scalar=1.0, in1=mv4,
                    op0=Alu.bypass, op1=Alu.mult,
                )
                # pairwise-add tree over k (in place, contiguous halves)
                k = kg
                while k > 1:
                    half = k // 2
                    lo = g[:, : half * TE]
                    hi = g[:, (k - half) * TE : k * TE]
                    nc.vector.scalar_tensor_tensor(
                        out=lo, in0=lo, scalar=1.0, in1=hi,
                        op0=Alu.bypass, op1=Alu.add,
                    )
                    k -= half
                if with_h0:
                    h0t = hpool.tile([128, TE], f16, tag="h0t")
                    nc.sync.dma_start(
                        out=h0t[:],
                        in_=h0sw_d[hoff[gi] : hoff[gi] + 128 * TE].rearrange(
                            "(p x) -> p x", x=TE
                        ),
                    )
                    nc.vector.scalar_tensor_tensor(
                        out=g[:, :TE], in0=g[:, :TE], scalar=1.0, in1=h0t[:],
                        op0=Alu.bypass, op1=Alu.add,
                    )
                nc.sync.dma_start(
                    out=dst_blk[t0 * 128 : (t0 + Tg) * 128, :].rearrange(
                        "(t p) e -> p t e", p=128
                    ),
                    in_=g[:, :TE].rearrange("p (t e) -> p t e", e=E),
                )
                if debug_taps:
                    dbg_t = dbg_h1 if lname == "m1" else dbg_cmp
                    nc.sync.dma_start(
                        out=dbg_t[t0 * 128 : (t0 + Tg) * 128, :].rearrange(
                            "(t p) e -> p t e", p=128
                        ),
                        in_=g[:, :TE].rearrange("p (t e) -> p t e", e=E),
                    )
                if post_group is not None:
                    post_group(gi, t0, Tg, pending)
            for fn in pending:
                fn()

        def _dummy_out():
            dummy = res.tile([B_LOC, E], f32, tag="dummy", name="dummy")
            nc.vector.memset(dummy[:], 0.0)
            nc.sync.dma_start(out=out_d, in_=dummy[:])

        # ---------------- layer 1 + chunked AllGather ----------------
        def l1_post(gi, t0, Tg, pending):
            if stage == "l1":
                return
            tend = t0 + Tg
            if tend % CHT == 0:
                ch = tend // CHT - 1

                def fire(ch=ch):
                    nc.gpsimd.collective_compute(
                        "AllGather",
                        Alu.bypass,
                        replica_groups=rg,
                        ins=[h1_blk[ch * CH : (ch + 1) * CH, :]],
                        outs=[h1_full[ch * NCORES * CH : (ch + 1) * NCORES * CH, :]],
                    )

                pending.append(fire)

        sparse_layer(groups1, moff1, pk1_d, pv1_d, emb_pos, h1_blk, "m1",
                     post_group=l1_post)
        done = stage == "l1"
        if done:
            _dummy_out()

        # ---------------- layer 2 (+ emb/3) + AllGather ----------------
        if not done:
            sparse_layer(groups2, moff2, pk2_d, pv2_d, h1_full, cmp_blk, "m2",
                         with_h0=True)
            nc.gpsimd.collective_compute(
                "AllGather",
                Alu.bypass,
                replica_groups=rg,
                ins=[cmp_blk[:]],
                outs=[cmp_full[:]],
            )
            if stage == "l2":
                _dummy_out()
                done = True

        if not done:
            # ---------------- stage 2: session attention ----------------
            ident = res.tile([128, 128], f32, tag="ident")
            make_identity(nc, ident[:])

            # weights
            w1t_t = res.tile([E, E], f32, tag="w1t")
            w1b_t = res.tile([E, E], f32, tag="w1b")
            g1w_t = res.tile([E, E], f32, tag="g1w")
            g1b_t = res.tile([E, 1], f32, tag="g1b")
            g2w_t = res.tile([E, E], f32, tag="g2w")
            w2_t = res.tile([E, 1], f32, tag="w2")
            pos_t = res.tile([L, E], f32, tag="pos")
            mask_t = res.tile([1, SESS], f32, tag="maskt")
            slen_t = res.tile([1, B_LOC], f32, tag="slent")
            for tt, dd in [
                (w1t_t, w1t_d), (w1b_t, w1b_d), (g1w_t, g1w_d), (g1b_t, g1b_d),
                (g2w_t, g2w_d), (w2_t, w2_d), (pos_t, pos_d), (mask_t, mask_d),
                (slen_t, slen_d),
            ]:
                nc.sync.dma_start(out=tt[:], in_=dd)

            sidx_t = res.tile([128, SJ], i32, tag="sidxt")
            nc.sync.dma_start(out=sidx_t[:], in_=sidx_d)
            g16 = res.tile([128, SJ * E], f16, tag="g16")
            for j in range(SJ):
                nc.gpsimd.indirect_dma_start(
                    out=g16[:, j * E : (j + 1) * E],
                    out_offset=None,
                    in_=cmp_full,
                    in_offset=bass.IndirectOffsetOnAxis(
                        ap=sidx_t[:, j : j + 1], axis=0
                    ),
                )
            g32 = res.tile([128, SJ * E], f32, tag="g32")
            nc.vector.tensor_copy(out=g32[:], in_=g16[:])

            seq_T = res.tile([128, SJ * 128], f32, tag="seqT")
            nc.vector.memset(seq_T[:], 0.0)
            nh_T = res.tile([E, SESS], f32, tag="nhT")
            nh2_T = res.tile([E, SESS], f32, tag="nh2T")
            beta_t = res.tile([1, SESS], f32, tag="betat")
            wsum = res.tile([128, SESS], f32, tag="wsum")
            hs_T = res.tile([128, B_LOC], f32, tag="hsT")
            pos_rep = res.tile([E, 10 * L], f32, tag="posrep")
            ones_t = res.tile([1, 128], f32, tag="ones")
            nc.vector.memset(ones_t[:], 1.0)

            with tc.tile_pool(name="psA", bufs=2, space="PSUM") as psA, \
                 tc.tile_pool(name="psB", bufs=2, space="PSUM") as psB, \
                 tc.tile_pool(name="psC", bufs=1, space="PSUM") as psC, \
                 tc.tile_pool(name="psD", bufs=1, space="PSUM") as psD, \
                 tc.tile_pool(name="psT", bufs=2, space="PSUM") as psT:
                # transposes: seq chunks, pos_T, glu1_wT, glu2_wT
                for j in range(SJ):
                    pt = psT.tile([128, 128], f32, tag="pt")
                    nc.tensor.transpose(
                        out=pt[:E, :], in_=g32[:, j * E : (j + 1) * E],
                        identity=ident[:],
                    )
                    nc.vector.tensor_copy(
                        out=seq_T[:E, j * 128 : (j + 1) * 128], in_=pt[:E, :]
                    )
                posT_t = res.tile([E, L], f32, tag="posT")
                pt = psT.tile([128, 128], f32, tag="pt")
                nc.tensor.transpose(out=pt[:E, :L], in_=pos_t[:], identity=ident[:L, :L])
                nc.vector.tensor_copy(out=posT_t[:], in_=pt[:E, :L])
                g1wT_t = res.tile([E, E], f32, tag="g1wT")
                pt = psT.tile([128, 128], f32, tag="pt")
                nc.tensor.transpose(out=pt[:E, :E], in_=g1w_t[:], identity=ident[:E, :E])
                nc.vector.tensor_copy(out=g1wT_t[:], in_=pt[:E, :E])
                g2wT_t = res.tile([E, E], f32, tag="g2wT")
                pt = psT.tile([128, 128], f32, tag="pt")
                nc.tensor.transpose(out=pt[:E, :E], in_=g2w_t[:], identity=ident[:E, :E])
                nc.vector.tensor_copy(out=g2wT_t[:], in_=pt[:E, :E])

                # pos_rep: pos_T columns repeated for 10 sessions
                nc.vector.tensor_copy(
                    out=pos_rep[:].rearrange("p (b l) -> p b l", l=L),
                    in_=posT_t[:].unsqueeze(1).broadcast_to([E, 10, L]),
                )

                # hs_T = (sum_l seq) / len
                hsum = res.tile([128, B_LOC], f32, tag="hsum")
                nc.vector.tensor_reduce(
                    out=hsum[:],
                    in_=seq_T[:, :SESS].rearrange("p (b l) -> p b l", l=L),
                    axis=X,
                    op=Alu.add,
                )
                rcp = res.tile([1, B_LOC], f32, tag="rcp")
                nc.vector.reciprocal(out=rcp[:], in_=slen_t[:])
                pr = psT.tile([128, B_LOC], f32, tag="pt")
                nc.tensor.matmul(out=pr[:], lhsT=ones_t[:], rhs=rcp[:], start=True, stop=True)
                nc.vector.tensor_tensor(out=hs_T[:], in0=hsum[:], in1=pr[:], op=Alu.mult)

                # session chunks of 10 sessions (500 cols)
                nb = 10
                for b0 in range(0, B_LOC, nb):
                    bn = min(nb, B_LOC - b0)
                    ch = bn * L
                    c0 = b0 * L
                    pA = psA.tile([E, nb * L], f32, tag="pA")
                    nc.tensor.matmul(
                        out=pA[:, :ch], lhsT=w1b_t[:], rhs=seq_T[:E, c0 : c0 + ch],
                        start=True, stop=False,
                    )
                    nc.tensor.matmul(
                        out=pA[:, :ch], lhsT=w1t_t[:], rhs=pos_rep[:, :ch],
                        start=False, stop=True,
                    )
                    nc.scalar.activation(out=nh_T[:, c0 : c0 + ch], in_=pA[:, :ch], func=Act.Tanh)

                    hs_rep = res.tile([E, nb * L], f32, tag="hsrep")
                    nc.vector.tensor_copy(
                        out=hs_rep[:, :ch].rearrange("p (b l) -> p b l", l=L),
                        in_=hs_T[:E, b0 : b0 + bn].unsqueeze(2).broadcast_to([E, bn, L]),
                    )
                    pB = psB.tile([E, nb * L], f32, tag="pB")
                    nc.tensor.matmul(
                        out=pB[:, :ch], lhsT=g1wT_t[:], rhs=nh_T[:, c0 : c0 + ch],
                        start=True, stop=False,
                    )
                    nc.tensor.matmul(
                        out=pB[:, :ch], lhsT=g2wT_t[:], rhs=hs_rep[:, :ch],
                        start=False, stop=True,
                    )
                    nc.scalar.activation(
                        out=nh2_T[:, c0 : c0 + ch], in_=pB[:, :ch], func=Act.Sigmoid,
                        bias=g1b_t[:],
                    )
                    pC = psC.tile([1, nb * L], f32, tag="pC")
                    nc.tensor.matmul(
                        out=pC[:, :ch], lhsT=w2_t[:], rhs=nh2_T[:, c0 : c0 + ch],
                        start=True, stop=True,
                    )
                    nc.vector.tensor_tensor(
                        out=beta_t[:, c0 : c0 + ch], in0=pC[:, :ch],
                        in1=mask_t[:, c0 : c0 + ch], op=Alu.mult,
                    )
                    pD = psD.tile([128, nb * L], f32, tag="pD")
                    nc.tensor.matmul(
                        out=pD[:, :ch], lhsT=ones_t[:], rhs=beta_t[:, c0 : c0 + ch],
                        start=True, stop=True,
                    )
                    nc.vector.tensor_tensor(
                        out=wsum[:, c0 : c0 + ch], in0=seq_T[:, c0 : c0 + ch],
                        in1=pD[:, :ch], op=Alu.mult,
                    )

                sel_T = res.tile([128, B_LOC], f32, tag="selT")
                nc.vector.tensor_reduce(
                    out=sel_T[:],
                    in_=wsum[:].rearrange("p (b l) -> p b l", l=L),
                    axis=X,
                    op=Alu.add,
                )
                po = psT.tile([128, 128], f32, tag="pt")
                nc.tensor.transpose(
                    out=po[:B_LOC, :], in_=sel_T[:], identity=ident[:]
                )
                outsb = res.tile([B_LOC, 128], f32, tag="outsb")
                nc.vector.tensor_copy(out=outsb[:], in_=po[:B_LOC, :])
                nc.sync.dma_start(out=out_d, in_=outsb[:, :E])

    nc.compile()
    return nc


# --------------------------------------------------------------------------
# entry point
# --------------------------------------------------------------------------
def kernel(**inputs):
    from concourse import bass_utils

    cfg, per_core = _preprocess(inputs)
    nc = _build_program(cfg)
    in_maps = [dict(pc) for pc in per_core]
    res = bass_utils.run_bass_kernel_spmd(
        nc, in_maps, core_ids=list(range(NCORES)), trace=False
    )
    out = np.concatenate([res.results[c]["out"] for c in range(NCORES)], axis=0)
    return out.astype(np.float32)


if __name__ == "__main__":
    pass


# revision 20
# speedup vs baseline: 1.0606x; 1.0606x over previous
"""COTREC GNN message-passing kernel for 8 Trainium2 NeuronCores.

Strategy (sharding_hint: row-shard sparse mm + all-gather; data-parallel
sessions):
  - Nodes are degree-sorted and striped across the 8 cores so every core gets
    an identical per-row-tile degree profile (one SPMD program).
  - The item table is stored fp16, unpadded (E=100 -> 200B rows), laid out in
    the chunk-major AllGather order.
  - Layer 1 (h1 = A @ emb): row tiles are batched into groups with a uniform
    neighbor-slot count k; ONE indirect DMA per group gathers all 128*k*Tg
    neighbor rows (SWDGE fixed cost ~1us amortizes over the whole group).
    DVE does one broadcast multiply by edge values and a log2(k) in-place
    pairwise-add tree (fp16 fast modes). h1 chunks are AllGathered (fp16),
    overlapped with compute.
  - Layer 2 (h2 = A @ h1) is computed only for nodes referenced by sessions
    (~23% of nodes); (emb + h1 + h2)/3 rows are packed into a compact fp16
    table and AllGathered.
  - Stage 2 (session attention pooling) is data-parallel over sessions (64
    per core) in feature-major layout: PE matmuls + ACT tanh/sigmoid + DVE.
Host side does only index preprocessing / sharding / packing (numpy).
"""
import sys

sys.path.insert(0, "/opt/trn_rl_repo")

import numpy as np

NCORES = 8
E = 100  # feature dim, rows stored unpadded fp16 (200B)


def _greedy_groups(ks, lo, hi, max_cols=240, waste=0.0):
    """Group consecutive tiles [lo,hi) (ks descending-ish) with uniform k.

    Returns list of (t0, Tg, kg). Waste = kg*Tg - sum(k) bounded."""
    out = []
    i = lo
    while i < hi:
        kmax = int(ks[i])
        sum_k = int(ks[i])
        j = i + 1
        while j < hi:
            km = max(kmax, int(ks[j]))
            n = j - i + 1
            s = sum_k + int(ks[j])
            if km * n > max_cols:
                break
            if km * n - s > waste * s:
                break
            kmax, sum_k = km, s
            j += 1
        out.append((i, j - i, kmax))
        i = j
    return out


# --------------------------------------------------------------------------
# host preprocessing
# --------------------------------------------------------------------------
def _preprocess(inputs):
    rows = np.asarray(inputs["adj_rows"]).astype(np.int64).ravel()
    cols = np.asarray(inputs["adj_cols"]).astype(np.int64).ravel()
    vals = np.asarray(inputs["adj_vals"]).astype(np.float32).ravel()
    emb = np.asarray(inputs["embedding"]).astype(np.float32)
    sess = np.asarray(inputs["reversed_sess_item"]).astype(np.int64)
    mask = np.asarray(inputs["mask"]).astype(np.float32)
    slen = np.asarray(inputs["session_len"]).astype(np.float32)
    posemb = np.asarray(inputs["pos_embedding"]).astype(np.float32)

    N, E_ = emb.shape
    assert E_ == E
    B, L = sess.shape
    assert B % NCORES == 0
    B_LOC = B // NCORES
    RLOC = -(-N // NCORES)
    T1 = -(-RLOC // 128)
    RPAD = T1 * 128
    if RLOC == RPAD:
        T1 += 1
        RPAD += 128

    # AllGather chunking: h1_full is chunk-major: [chunk][rank][rows-in-chunk]
    nch = next(d for d in (7, 8, 6, 5, 4, 3, 2, 1) if T1 % d == 0)
    CHT = T1 // nch  # tiles per chunk
    CH = CHT * 128  # rows per chunk per core

    deg = np.bincount(rows, minlength=N).astype(np.int64)
    order = np.argsort(-deg, kind="stable")
    pos_of = np.empty(N, np.int64)
    pos_of[order] = np.arange(N)
    deg_sorted = deg[order]
    # table position of node n in the chunk-major AG layout
    _c = pos_of % NCORES
    _s = pos_of // NCORES
    tab_of = (_s // CH) * (NCORES * CH) + _c * CH + (_s % CH)

    # per-tile max degree (stripe of 128*NCORES sorted nodes)
    stripe_starts = np.minimum(np.arange(T1) * (128 * NCORES), N - 1)
    k1 = np.maximum(1, deg_sorted[stripe_starts]).astype(np.int64)

    # groups within AG chunks
    groups1 = []
    for c in range(nch):
        groups1 += _greedy_groups(k1, c * CHT, (c + 1) * CHT)
    g_of_tile1 = np.empty(T1, np.int64)
    moff1 = np.empty(len(groups1), np.int64)
    off = 0
    for gi, (t0, Tg, kg) in enumerate(groups1):
        g_of_tile1[t0 : t0 + Tg] = gi
        moff1[gi] = off
        off += 128 * kg * Tg
    S1G = off

    # edge -> (core, tile, partition, j)
    p_e = pos_of[rows]
    c_e = (p_e % NCORES).astype(np.int64)
    loc_e = p_e // NCORES
    ordE = np.argsort(p_e, kind="stable")
    pe_s = p_e[ordE]
    j_s = np.arange(len(rows)) - np.searchsorted(pe_s, pe_s, side="left")
    j_e = np.empty(len(rows), np.int64)
    j_e[ordE] = j_s
    t_e = loc_e // 128
    part_e = loc_e % 128
    # meta layout per group: [p][j*Tg + (t - t0)] (k-major columns)
    g_e = g_of_tile1[t_e]
    t0_e = np.array([g[0] for g in groups1])[g_e]
    Tg_e = np.array([g[1] for g in groups1])[g_e]
    kg_e = np.array([g[2] for g in groups1])[g_e]
    mpos = moff1[g_e] + part_e * (kg_e * Tg_e) + j_e * Tg_e + (t_e - t0_e)
    col_tab = tab_of[cols].astype(np.int32)
    pk1 = np.zeros((NCORES, S1G), np.int32)
    pv1 = np.zeros((NCORES, S1G), np.float16)
    pk1[c_e, mpos] = col_tab
    pv1[c_e, mpos] = vals.astype(np.float16)

    # positioned fp16 table (zero rows for pad positions)
    emb_pos = np.zeros((NCORES * RPAD, E), np.float16)
    emb_pos[tab_of[np.arange(N)]] = emb.astype(np.float16)

    # ---------------- layer 2 (needed nodes only) ----------------
    s_nodes = np.unique(sess[sess > 0]).astype(np.int64) - 1
    s_sorted = s_nodes[np.argsort(-deg[s_nodes], kind="stable")]
    NS = len(s_sorted)
    R2 = -(-NS // NCORES)
    T2 = -(-R2 // 128)
    R2P = T2 * 128
    used0 = -(-NS // NCORES)  # occupied local slots on core 0
    if used0 >= R2P:  # need a free (all-zero) pad slot for idx==0 sessions
        T2 += 1
        R2P += 128
    spos = np.full(N, -1, np.int64)
    spos[s_sorted] = np.arange(NS)

    cs2 = np.minimum(np.arange(T2) * (128 * NCORES), NS - 1)
    k2 = (np.maximum(1, deg[s_sorted[cs2]]) + 1).astype(np.int64)  # +1 self

    groups2 = _greedy_groups(k2, 0, T2)
    g_of_tile2 = np.empty(T2, np.int64)
    moff2 = np.empty(len(groups2), np.int64)
    off = 0
    for gi, (t0, Tg, kg) in enumerate(groups2):
        g_of_tile2[t0 : t0 + Tg] = gi
        moff2[gi] = off
        off += 128 * kg * Tg
    S2G = off

    pk2 = np.zeros((NCORES, S2G), np.int32)
    pv2 = np.zeros((NCORES, S2G), np.float16)
    # self slot j=0: + h1[i]/3
    q = np.arange(NS)
    qc = q % NCORES
    qs = q // NCORES
    qt = qs // 128
    qp = qs % 128
    g_q = g_of_tile2[qt]
    t0_q = np.array([g[0] for g in groups2])[g_q]
    Tg_q = np.array([g[1] for g in groups2])[g_q]
    kg_q = np.array([g[2] for g in groups2])[g_q]
    mq = moff2[g_q] + qp * (kg_q * Tg_q) + 0 * Tg_q + (qt - t0_q)
    pk2[qc, mq] = tab_of[s_sorted].astype(np.int32)
    pv2[qc, mq] = np.float16(1.0 / 3.0)
    # edges with row in S, slots j>=1
    maskE = spos[rows] >= 0
    q_e2 = spos[rows[maskE]]
    ordE2 = np.argsort(q_e2, kind="stable")
    qe_s = q_e2[ordE2]
    j2_s = np.arange(len(qe_s)) - np.searchsorted(qe_s, qe_s, side="left")
    j2 = np.empty(len(qe_s), np.int64)
    j2[ordE2] = j2_s
    c2 = q_e2 % NCORES
    s2 = q_e2 // NCORES
    t2_ = s2 // 128
    p2 = s2 % 128
    g_2 = g_of_tile2[t2_]
    t0_2 = np.array([g[0] for g in groups2])[g_2]
    Tg_2 = np.array([g[1] for g in groups2])[g_2]
    kg_2 = np.array([g[2] for g in groups2])[g_2]
    mpos2 = moff2[g_2] + p2 * (kg_2 * Tg_2) + (1 + j2) * Tg_2 + (t2_ - t0_2)
    pk2[c2, mpos2] = col_tab[maskE]
    pv2[c2, mpos2] = (vals[maskE] / 3.0).astype(np.float16)

    # host-gathered emb/3 rows, pre-swizzled per group: flat [p][t][e]
    h0rows = np.zeros((NCORES, R2P, E), np.float16)
    h0rows[qc, qs] = (emb[s_sorted] / 3.0).astype(np.float16)
    h0sw = np.zeros((NCORES, T2 * 128 * E), np.float16)
    hoff = {}
    off = 0
    for gi, (t0, Tg, kg) in enumerate(groups2):
        hoff[gi] = off
        blk = h0rows[:, t0 * 128 : (t0 + Tg) * 128, :]  # [NC, Tg*128, E]
        blk = blk.reshape(NCORES, Tg, 128, E).transpose(0, 2, 1, 3)  # [NC,p,t,e]
        h0sw[:, off : off + 128 * Tg * E] = blk.reshape(NCORES, -1)
        off += 128 * Tg * E
    h0sw = h0sw[:, :off]
    H0SZ = off

    # ---------------- session routing ----------------
    pad_crow = 0 * R2P + used0  # all-zero row in compact table (core0 pad)
    crow = np.full((B, L), pad_crow, np.int64)
    nz = sess > 0
    qv = spos[sess[nz] - 1]
    assert (qv >= 0).all()
    crow[nz] = (qv % NCORES) * R2P + qv // NCORES
    SESS = B_LOC * L
    SJ = -(-SESS // 128)
    assert SJ * 128 == SESS, "SESS must be a multiple of 128"
    sidx = np.full((NCORES, 128, SJ), pad_crow, np.int32)
    for c in range(NCORES):
        flat = crow[c * B_LOC : (c + 1) * B_LOC].ravel()  # r = b_loc*L + l
        rr = np.arange(SESS)
        sidx[c, rr % 128, rr // 128] = flat

    mask_c = mask.reshape(NCORES, 1, SESS).astype(np.float32)
    slen_c = slen.reshape(NCORES, 1, B_LOC).astype(np.float32)

    cfg = dict(
        N=N, B=B, L=L, B_LOC=B_LOC, RLOC=RLOC, RPAD=RPAD, T1=T1,
        T2=T2, R2P=R2P, S1G=S1G, S2G=S2G, H0SZ=H0SZ, SJ=SJ, SESS=SESS,
        NCH=nch, CHT=CHT,
        groups1=groups1, groups2=groups2,
        moff1=[int(x) for x in moff1], moff2=[int(x) for x in moff2],
        hoff=hoff,
    )
    per_core = []
    for c in range(NCORES):
        per_core.append(
            dict(
                emb_pos=emb_pos,
                pk1=pk1[c], pv1=pv1[c],
                pk2=pk2[c], pv2=pv2[c],
                h0sw=h0sw[c],
                sidx=sidx[c],
                pos50=np.ascontiguousarray(posemb[:L]),
                w1t=np.ascontiguousarray(np.asarray(inputs["w_1"])[:E].astype(np.float32)),
                w1b=np.ascontiguousarray(np.asarray(inputs["w_1"])[E:].astype(np.float32)),
                g1w=np.asarray(inputs["glu1_w"]).astype(np.float32),
                g1b=np.asarray(inputs["glu1_b"]).astype(np.float32).reshape(E, 1),
                g2w=np.asarray(inputs["glu2_w"]).astype(np.float32),
                w2=np.asarray(inputs["w_2"]).astype(np.float32),
                mask_c=mask_c[c],
                slen_c=slen_c[c],
            )
        )
    return cfg, per_core


# --------------------------------------------------------------------------
# device program
# --------------------------------------------------------------------------
def _build_program(cfg, stage="full", debug_taps=False):
    import concourse.bass as bass
    import concourse.bacc as bacc
    import concourse.mybir as mybir
    import concourse.tile as tile
    from concourse.masks import make_identity

    dt = mybir.dt
    f32, f16, i32 = dt.float32, dt.float16, dt.int32
    Alu = mybir.AluOpType
    Act = mybir.ActivationFunctionType
    X = mybir.AxisListType.X

    L = cfg["L"]
    B_LOC = cfg["B_LOC"]
    RPAD = cfg["RPAD"]
    T2 = cfg["T2"]
    R2P = cfg["R2P"]
    S1G = cfg["S1G"]
    S2G = cfg["S2G"]
    H0SZ = cfg["H0SZ"]
    SJ = cfg["SJ"]
    SESS = cfg["SESS"]
    groups1 = cfg["groups1"]
    groups2 = cfg["groups2"]
    moff1 = cfg["moff1"]
    moff2 = cfg["moff2"]
    hoff = cfg["hoff"]
    NCH = cfg["NCH"]
    CHT = cfg["CHT"]
    CH = CHT * 128
    rg = [list(range(NCORES))]

    nc = bacc.Bacc(
        "TRN2", target_bir_lowering=False, debug=False, num_devices=NCORES
    )
    emb_pos = nc.dram_tensor("emb_pos", [NCORES * RPAD, E], f16, kind="ExternalInput").ap()
    pk1_d = nc.dram_tensor("pk1", [S1G], i32, kind="ExternalInput").ap()
    pv1_d = nc.dram_tensor("pv1", [S1G], f16, kind="ExternalInput").ap()
    pk2_d = nc.dram_tensor("pk2", [S2G], i32, kind="ExternalInput").ap()
    pv2_d = nc.dram_tensor("pv2", [S2G], f16, kind="ExternalInput").ap()
    h0sw_d = nc.dram_tensor("h0sw", [H0SZ], f16, kind="ExternalInput").ap()
    sidx_d = nc.dram_tensor("sidx", [128, SJ], i32, kind="ExternalInput").ap()
    pos_d = nc.dram_tensor("pos50", [L, E], f32, kind="ExternalInput").ap()
    w1t_d = nc.dram_tensor("w1t", [E, E], f32, kind="ExternalInput").ap()
    w1b_d = nc.dram_tensor("w1b", [E, E], f32, kind="ExternalInput").ap()
    g1w_d = nc.dram_tensor("g1w", [E, E], f32, kind="ExternalInput").ap()
    g1b_d = nc.dram_tensor("g1b", [E, 1], f32, kind="ExternalInput").ap()
    g2w_d = nc.dram_tensor("g2w", [E, E], f32, kind="ExternalInput").ap()
    w2_d = nc.dram_tensor("w2", [E, 1], f32, kind="ExternalInput").ap()
    mask_d = nc.dram_tensor("mask_c", [1, SESS], f32, kind="ExternalInput").ap()
    slen_d = nc.dram_tensor("slen_c", [1, B_LOC], f32, kind="ExternalInput").ap()
    out_d = nc.dram_tensor("out", [B_LOC, E], f32, kind="ExternalOutput").ap()

    if debug_taps:
        g1_0 = cfg["groups1"][0]
        dbg_g = nc.dram_tensor(
            "dbg_g", [128, g1_0[1] * g1_0[2] * E], f16, kind="ExternalOutput"
        ).ap()
        dbg_h1 = nc.dram_tensor("dbg_h1", [RPAD, E], f16, kind="ExternalOutput").ap()
        dbg_cmp = nc.dram_tensor("dbg_cmp", [R2P, E], f16, kind="ExternalOutput").ap()
    h1_blk = nc.dram_tensor("h1_blk", [RPAD, E], f16, kind="Internal").ap()
    h1_full = nc.dram_tensor(
        "h1_full", [NCORES * RPAD, E], f16, kind="Internal", addr_space="Shared"
    ).ap()
    cmp_blk = nc.dram_tensor("cmp_blk", [R2P, E], f16, kind="Internal").ap()
    cmp_full = nc.dram_tensor(
        "cmp_full", [NCORES * R2P, E], f16, kind="Internal", addr_space="Shared"
    ).ap()

    from contextlib import ExitStack

    with tile.TileContext(nc) as tc, ExitStack() as ctx:
        res = ctx.enter_context(tc.tile_pool(name="res", bufs=1))
        mpool = ctx.enter_context(tc.tile_pool(name="meta", bufs=2))
        gpool = ctx.enter_context(tc.tile_pool(name="g", bufs=2))
        hpool = ctx.enter_context(tc.tile_pool(name="h0", bufs=2))

        def sparse_layer(groups, moff, pk_d, pv_d, src_tab, dst_blk, lname,
                         with_h0=False, post_group=None):
            pending = []  # delayed actions (AG triggers), flushed mid-gather
            for gi, (t0, Tg, kg) in enumerate(groups):
                Kg = kg * Tg
                TE = Tg * E
                mi = mpool.tile([128, Kg], i32, tag=f"{lname}i", name=f"{lname}i_{gi}")
                mv = mpool.tile([128, Kg], f16, tag=f"{lname}v", name=f"{lname}v_{gi}")
                nc.sync.dma_start(
                    out=mi[:],
                    in_=pk_d[moff[gi] : moff[gi] + 128 * Kg].rearrange(
                        "(p c) -> p c", c=Kg
                    ),
                )
                nc.sync.dma_start(
                    out=mv[:],
                    in_=pv_d[moff[gi] : moff[gi] + 128 * Kg].rearrange(
                        "(p c) -> p c", c=Kg
                    ),
                )
                g = gpool.tile([128, Kg * E], f16, tag="g")
                # one indirect DMA per slot column (the only offset/run pairing
                # the SWDGE ucode supports: one offset per partition)
                flush_at = Kg // 2
                for c in range(Kg):
                    if c == flush_at:
                        for fn in pending:
                            fn()
                        pending = []
                    nc.gpsimd.indirect_dma_start(
                        out=g[:, c * E : (c + 1) * E],
                        out_offset=None,
                        in_=src_tab,
                        in_offset=bass.IndirectOffsetOnAxis(
                            ap=mi[:, c : c + 1], axis=0
                        ),
                    )
                if debug_taps and lname == "m1" and gi == 0:
                    nc.sync.dma_start(out=dbg_g, in_=g[:])
                # multiply every gathered row by its edge value (k-major
                # layout: column block c = j*Tg + t, value per (j, t))
                g4 = g[:].rearrange("p (k t e) -> p k t e", t=Tg, e=E)
                mv4 = (
                    mv[:]
                    .rearrange("p (k t) -> p k t", t=Tg)
                    .unsqueeze(3)
                    .broadcast_to([128, kg, Tg, E])
                )
                nc.vector.scalar_tensor_tensor(
                    out=g4, in0=g4, scalar=1.0, in1=mv4,
                    op0=Alu.bypass, op1=Alu.mult,
                )
                # pairwise-add tree over k (in place, contiguous halves)
                k = kg
                while k > 1:
                    half = k // 2
                    lo = g[:, : half * TE]
                    hi = g[:, (k - half) * TE : k * TE]
                    nc.vector.scalar_tensor_tensor(
                        out=lo, in0=lo, scalar=1.0, in1=hi,
                        op0=Alu.bypass, op1=Alu.add,
                    )
                    k -= half
                if with_h0:
                    h0t = hpool.tile([128, TE], f16, tag="h0t")
                    nc.sync.dma_start(
                        out=h0t[:],
                        in_=h0sw_d[hoff[gi] : hoff[gi] + 128 * TE].rearrange(
                            "(p x) -> p x", x=TE
                        ),
                    )
                    nc.vector.scalar_tensor_tensor(
                        out=g[:, :TE], in0=g[:, :TE], scalar=1.0, in1=h0t[:],
                        op0=Alu.bypass, op1=Alu.add,
                    )
                nc.sync.dma_start(
                    out=dst_blk[t0 * 128 : (t0 + Tg) * 128, :].rearrange(
                        "(t p) e -> p t e", p=128
                    ),
                    in_=g[:, :TE].rearrange("p (t e) -> p t e", e=E),
                )
                if debug_taps:
                    dbg_t = dbg_h1 if lname == "m1" else dbg_cmp
                    nc.sync.dma_start(
                        out=dbg_t[t0 * 128 : (t0 + Tg) * 128, :].rearrange(
                            "(t p) e -> p t e", p=128
                        ),
                        in_=g[:, :TE].rearrange("p (t e) -> p t e", e=E),
                    )
                if post_group is not None:
                    post_group(gi, t0, Tg, pending)
            for fn in pending:
                fn()

        def _dummy_out():
            dummy = res.tile([B_LOC, E], f32, tag="dummy", name="dummy")
            nc.vector.memset(dummy[:], 0.0)
            nc.sync.dma_start(out=out_d, in_=dummy[:])

        # ---------------- layer 1 + chunked AllGather ----------------
        def l1_post(gi, t0, Tg, pending):
            if stage == "l1":
                return
            tend = t0 + Tg
            if tend % CHT == 0:
                ch = tend // CHT - 1

                def fire(ch=ch):
                    nc.gpsimd.collective_compute(
                        "AllGather",
                        Alu.bypass,
                        replica_groups=rg,
                        ins=[h1_blk[ch * CH : (ch + 1) * CH, :]],
                        outs=[h1_full[ch * NCORES * CH : (ch + 1) * NCORES * CH, :]],
                    )

                pending.append(fire)

        sparse_layer(groups1, moff1, pk1_d, pv1_d, emb_pos, h1_blk, "m1",
                     post_group=l1_post)
        done = stage == "l1"
        if done:
            _dummy_out()

        # ---------------- layer 2 (+ emb/3) + AllGather ----------------
        if not done:
            sparse_layer(groups2, moff2, pk2_d, pv2_d, h1_full, cmp_blk, "m2",
                         with_h0=True)
            nc.gpsimd.collective_compute(
                "AllGather",
                Alu.bypass,
                replica_groups=rg,
                ins=[cmp_blk[:]],
                outs=[cmp_full[:]],
            )
            if stage == "l2":
                _dummy_out()
                done = True

        if not done:
            # ---------------- stage 2: session attention ----------------
            ident = res.tile([128, 128], f32, tag="ident")
            make_identity(nc, ident[:])

            # weights
            w1t_t = res.tile([E, E], f32, tag="w1t")
            w1b_t = res.tile([E, E], f32, tag="w1b")
            g1w_t = res.tile([E, E], f32, tag="g1w")
            g1b_t = res.tile([E, 1], f32, tag="g1b")
            g2w_t = res.tile([E, E], f32, tag="g2w")
            w2_t = res.tile([E, 1], f32, tag="w2")
            pos_t = res.tile([L, E], f32, tag="pos")
            mask_t = res.tile([1, SESS], f32, tag="maskt")
            slen_t = res.tile([1, B_LOC], f32, tag="slent")
            for tt, dd in [
                (w1t_t, w1t_d), (w1b_t, w1b_d), (g1w_t, g1w_d), (g1b_t, g1b_d),
                (g2w_t, g2w_d), (w2_t, w2_d), (pos_t, pos_d), (mask_t, mask_d),
                (slen_t, slen_d),
            ]:
                nc.sync.dma_start(out=tt[:], in_=dd)

            sidx_t = res.tile([128, SJ], i32, tag="sidxt")
            nc.sync.dma_start(out=sidx_t[:], in_=sidx_d)
            g16 = res.tile([128, SJ * E], f16, tag="g16")
            for j in range(SJ):
                nc.gpsimd.indirect_dma_start(
                    out=g16[:, j * E : (j + 1) * E],
                    out_offset=None,
                    in_=cmp_full,
                    in_offset=bass.IndirectOffsetOnAxis(
                        ap=sidx_t[:, j : j + 1], axis=0
                    ),
                )
            g32 = res.tile([128, SJ * E], f32, tag="g32")
            nc.vector.tensor_copy(out=g32[:], in_=g16[:])

            seq_T = res.tile([128, SJ * 128], f32, tag="seqT")
            nc.vector.memset(seq_T[:], 0.0)
            nh_T = res.tile([E, SESS], f32, tag="nhT")
            nh2_T = res.tile([E, SESS], f32, tag="nh2T")
            beta_t = res.tile([1, SESS], f32, tag="betat")
            wsum = res.tile([128, SESS], f32, tag="wsum")
            hs_T = res.tile([128, B_LOC], f32, tag="hsT")
            pos_rep = res.tile([E, 10 * L], f32, tag="posrep")
            ones_t = res.tile([1, 128], f32, tag="ones")
            nc.vector.memset(ones_t[:], 1.0)

            with tc.tile_pool(name="psA", bufs=2, space="PSUM") as psA, \
                 tc.tile_pool(name="psB", bufs=2, space="PSUM") as psB, \
                 tc.tile_pool(name="psC", bufs=1, space="PSUM") as psC, \
                 tc.tile_pool(name="psD", bufs=1, space="PSUM") as psD, \
                 tc.tile_pool(name="psT", bufs=2, space="PSUM") as psT:
                # transposes: seq chunks, pos_T, glu1_wT, glu2_wT
                for j in range(SJ):
                    pt = psT.tile([128, 128], f32, tag="pt")
                    nc.tensor.transpose(
                        out=pt[:E, :], in_=g32[:, j * E : (j + 1) * E],
                        identity=ident[:],
                    )
                    nc.vector.tensor_copy(
                        out=seq_T[:E, j * 128 : (j + 1) * 128], in_=pt[:E, :]
                    )
                posT_t = res.tile([E, L], f32, tag="posT")
                pt = psT.tile([128, 128], f32, tag="pt")
                nc.tensor.transpose(out=pt[:E, :L], in_=pos_t[:], identity=ident[:L, :L])
                nc.vector.tensor_copy(out=posT_t[:], in_=pt[:E, :L])
                g1wT_t = res.tile([E, E], f32, tag="g1wT")
                pt = psT.tile([128, 128], f32, tag="pt")
                nc.tensor.transpose(out=pt[:E, :E], in_=g1w_t[:], identity=ident[:E, :E])
                nc.vector.tensor_copy(out=g1wT_t[:], in_=pt[:E, :E])
                g2wT_t = res.tile([E, E], f32, tag="g2wT")
                pt = psT.tile([128, 128], f32, tag="pt")
                nc.tensor.transpose(out=pt[:E, :E], in_=g2w_t[:], identity=ident[:E, :E])
                nc.vector.tensor_copy(out=g2wT_t[:], in_=pt[:E, :E])

                # pos_rep: pos_T columns repeated for 10 sessions
                nc.vector.tensor_copy(
                    out=pos_rep[:].rearrange("p (b l) -> p b l", l=L),
                    in_=posT_t[:].unsqueeze(1).broadcast_to([E, 10, L]),
                )

                # hs_T = (sum_l seq) / len
                hsum = res.tile([128, B_LOC], f32, tag="hsum")
                nc.vector.tensor_reduce(
                    out=hsum[:],
                    in_=seq_T[:, :SESS].rearrange("p (b l) -> p b l", l=L),
                    axis=X,
                    op=Alu.add,
                )
                rcp = res.tile([1, B_LOC], f32, tag="rcp")
                nc.vector.reciprocal(out=rcp[:], in_=slen_t[:])
                pr = psT.tile([128, B_LOC], f32, tag="pt")
                nc.tensor.matmul(out=pr[:], lhsT=ones_t[:], rhs=rcp[:], start=True, stop=True)
                nc.vector.tensor_tensor(out=hs_T[:], in0=hsum[:], in1=pr[:], op=Alu.mult)

                # session chunks of 10 sessions (500 cols)
                nb = 10
                for b0 in range(0, B_LOC, nb):
                    bn = min(nb, B_LOC - b0)
                    ch = bn * L
                    c0 = b0 * L
                    pA = psA.tile([E, nb * L], f32, tag="pA")
                    nc.tensor.matmul(
                        out=pA[:, :ch], lhsT=w1b_t[:], rhs=seq_T[:E, c0 : c0 + ch],
                        start=True, stop=False,
                    )
                    nc.tensor.matmul(
                        out=pA[:, :ch], lhsT=w1t_t[:], rhs=pos_rep[:, :ch],
                        start=False, stop=True,
                    )
                    nc.scalar.activation(out=nh_T[:, c0 : c0 + ch], in_=pA[:, :ch], func=Act.Tanh)

                    hs_rep = res.tile([E, nb * L], f32, tag="hsrep")
                    nc.vector.tensor_copy(
                        out=hs_rep[:, :ch].rearrange("p (b l) -> p b l", l=L),
                        in_=hs_T[:E, b0 : b0 + bn].unsqueeze(2).broadcast_to([E, bn, L]),
                    )
                    pB = psB.tile([E, nb * L], f32, tag="pB")
                    nc.tensor.matmul(
                        out=pB[:, :ch], lhsT=g1wT_t[:], rhs=nh_T[:, c0 : c0 + ch],
                        start=True, stop=False,
                    )
                    nc.tensor.matmul(
                        out=pB[:, :ch], lhsT=g2wT_t[:], rhs=hs_rep[:, :ch],
                        start=False, stop=True,
                    )
                    nc.scalar.activation(
                        out=nh2_T[:, c0 : c0 + ch], in_=pB[:, :ch], func=Act.Sigmoid,
                        bias=g1b_t[:],
                    )
                    pC = psC.tile([1, nb * L], f32, tag="pC")
                    nc.tensor.matmul(
                        out=pC[:, :ch], lhsT=w2_t[:], rhs=nh2_T[:, c0 : c0 + ch],
                        start=True, stop=True,
                    )
                    nc.vector.tensor_tensor(
                        out=beta_t[:, c0 : c0 + ch], in0=pC[:, :ch],
                        in1=mask_t[:, c0 : c0 + ch], op=Alu.mult,
                    )
                    pD = psD.tile([128, nb * L], f32, tag="pD")
                    nc.tensor.matmul(
                        out=pD[:, :ch], lhsT=ones_t[:], rhs=beta_t[:, c0 : c0 + ch],
                        start=True, stop=True,
                    )
                    nc.vector.tensor_tensor(
                        out=wsum[:, c0 : c0 + ch], in0=seq_T[:, c0 : c0 + ch],
                        in1=pD[:, :ch], op=Alu.mult,
                    )

                sel_T = res.tile([128, B_LOC], f32, tag="selT")
                nc.vector.tensor_reduce(
                    out=sel_T[:],
                    in_=wsum[:].rearrange("p (b l) -> p b l", l=L),
                    axis=X,
                    op=Alu.add,
                )
                po = psT.tile([128, 128], f32, tag="pt")
                nc.tensor.transpose(
                    out=po[:B_LOC, :], in_=sel_T[:], identity=ident[:]
                )
                outsb = res.tile([B_LOC, 128], f32, tag="outsb")
                nc.vector.tensor_copy(out=outsb[:], in_=po[:B_LOC, :])
                nc.sync.dma_start(out=out_d, in_=outsb[:, :E])

    nc.compile()
    return nc


# --------------------------------------------------------------------------
# entry point
# --------------------------------------------------------------------------
def kernel(**inputs):
    from concourse import bass_utils

    cfg, per_core = _preprocess(inputs)
    nc = _build_program(cfg)
    in_maps = [dict(pc) for pc in per_core]
    res = bass_utils.run_bass_kernel_spmd(
        nc, in_maps, core_ids=list(range(NCORES)), trace=False
    )
    out = np.concatenate([res.results[c]["out"] for c in range(NCORES)], axis=0)
    return out.astype(np.float32)


if __name__ == "__main__":
    pass


# revision 21
# speedup vs baseline: 1.0621x; 1.0014x over previous
"""COTREC GNN message-passing kernel for 8 Trainium2 NeuronCores.

Strategy (sharding_hint: row-shard sparse mm + all-gather; data-parallel
sessions):
  - Nodes are degree-sorted and striped across the 8 cores so every core gets
    an identical per-row-tile degree profile (one SPMD program).
  - The item table is stored fp16, unpadded (E=100 -> 200B rows), laid out in
    the chunk-major AllGather order.
  - Layer 1 (h1 = A @ emb): row tiles are batched into groups with a uniform
    neighbor-slot count k; ONE indirect DMA per group gathers all 128*k*Tg
    neighbor rows (SWDGE fixed cost ~1us amortizes over the whole group).
    DVE does one broadcast multiply by edge values and a log2(k) in-place
    pairwise-add tree (fp16 fast modes). h1 chunks are AllGathered (fp16),
    overlapped with compute.
  - Layer 2 (h2 = A @ h1) is computed only for nodes referenced by sessions
    (~23% of nodes); (emb + h1 + h2)/3 rows are packed into a compact fp16
    table and AllGathered.
  - Stage 2 (session attention pooling) is data-parallel over sessions (64
    per core) in feature-major layout: PE matmuls + ACT tanh/sigmoid + DVE.
Host side does only index preprocessing / sharding / packing (numpy).
"""
import sys

sys.path.insert(0, "/opt/trn_rl_repo")

import numpy as np

NCORES = 8
E = 100  # feature dim, rows stored unpadded fp16 (200B)


def _greedy_groups(ks, lo, hi, max_cols=240, waste=0.0):
    """Group consecutive tiles [lo,hi) (ks descending-ish) with uniform k.

    Returns list of (t0, Tg, kg). Waste = kg*Tg - sum(k) bounded."""
    out = []
    i = lo
    while i < hi:
        kmax = int(ks[i])
        sum_k = int(ks[i])
        j = i + 1
        while j < hi:
            km = max(kmax, int(ks[j]))
            n = j - i + 1
            s = sum_k + int(ks[j])
            if km * n > max_cols:
                break
            if km * n - s > waste * s:
                break
            kmax, sum_k = km, s
            j += 1
        out.append((i, j - i, kmax))
        i = j
    return out


# --------------------------------------------------------------------------
# host preprocessing
# --------------------------------------------------------------------------
def _preprocess(inputs):
    rows = np.asarray(inputs["adj_rows"]).astype(np.int64).ravel()
    cols = np.asarray(inputs["adj_cols"]).astype(np.int64).ravel()
    vals = np.asarray(inputs["adj_vals"]).astype(np.float32).ravel()
    emb = np.asarray(inputs["embedding"]).astype(np.float32)
    sess = np.asarray(inputs["reversed_sess_item"]).astype(np.int64)
    mask = np.asarray(inputs["mask"]).astype(np.float32)
    slen = np.asarray(inputs["session_len"]).astype(np.float32)
    posemb = np.asarray(inputs["pos_embedding"]).astype(np.float32)

    N, E_ = emb.shape
    assert E_ == E
    B, L = sess.shape
    assert B % NCORES == 0
    B_LOC = B // NCORES
    RLOC = -(-N // NCORES)
    T1 = -(-RLOC // 128)
    RPAD = T1 * 128
    if RLOC == RPAD:
        T1 += 1
        RPAD += 128

    # AllGather chunking: h1_full is chunk-major: [chunk][rank][rows-in-chunk]
    nch = next(d for d in (7, 8, 6, 5, 4, 3, 2, 1) if T1 % d == 0)
    CHT = T1 // nch  # tiles per chunk
    CH = CHT * 128  # rows per chunk per core

    deg = np.bincount(rows, minlength=N).astype(np.int64)
    order = np.argsort(-deg, kind="stable")
    pos_of = np.empty(N, np.int64)
    pos_of[order] = np.arange(N)
    deg_sorted = deg[order]
    # table position of node n in the chunk-major AG layout
    _c = pos_of % NCORES
    _s = pos_of // NCORES
    tab_of = (_s // CH) * (NCORES * CH) + _c * CH + (_s % CH)

    # per-tile max degree (stripe of 128*NCORES sorted nodes)
    stripe_starts = np.minimum(np.arange(T1) * (128 * NCORES), N - 1)
    k1 = np.maximum(1, deg_sorted[stripe_starts]).astype(np.int64)

    # groups within AG chunks
    groups1 = []
    for c in range(nch):
        groups1 += _greedy_groups(k1, c * CHT, (c + 1) * CHT)
    g_of_tile1 = np.empty(T1, np.int64)
    moff1 = np.empty(len(groups1), np.int64)
    off = 0
    for gi, (t0, Tg, kg) in enumerate(groups1):
        g_of_tile1[t0 : t0 + Tg] = gi
        moff1[gi] = off
        off += 128 * kg * Tg
    S1G = off

    # edge -> (core, tile, partition, j)
    p_e = pos_of[rows]
    c_e = (p_e % NCORES).astype(np.int64)
    loc_e = p_e // NCORES
    ordE = np.argsort(p_e, kind="stable")
    pe_s = p_e[ordE]
    j_s = np.arange(len(rows)) - np.searchsorted(pe_s, pe_s, side="left")
    j_e = np.empty(len(rows), np.int64)
    j_e[ordE] = j_s
    t_e = loc_e // 128
    part_e = loc_e % 128
    # meta layout per group: [p][j*Tg + (t - t0)] (k-major columns)
    g_e = g_of_tile1[t_e]
    t0_e = np.array([g[0] for g in groups1])[g_e]
    Tg_e = np.array([g[1] for g in groups1])[g_e]
    kg_e = np.array([g[2] for g in groups1])[g_e]
    mpos = moff1[g_e] + part_e * (kg_e * Tg_e) + j_e * Tg_e + (t_e - t0_e)
    col_tab = tab_of[cols].astype(np.int32)
    pk1 = np.zeros((NCORES, S1G), np.int32)
    pv1 = np.zeros((NCORES, S1G), np.float16)
    pk1[c_e, mpos] = col_tab
    pv1[c_e, mpos] = vals.astype(np.float16)

    # positioned fp16 table (zero rows for pad positions)
    emb_pos = np.zeros((NCORES * RPAD, E), np.float16)
    emb_pos[tab_of[np.arange(N)]] = emb.astype(np.float16)

    # ---------------- layer 2 (needed nodes only) ----------------
    s_nodes = np.unique(sess[sess > 0]).astype(np.int64) - 1
    s_sorted = s_nodes[np.argsort(-deg[s_nodes], kind="stable")]
    NS = len(s_sorted)
    R2 = -(-NS // NCORES)
    T2 = -(-R2 // 128)
    R2P = T2 * 128
    used0 = -(-NS // NCORES)  # occupied local slots on core 0
    if used0 >= R2P:  # need a free (all-zero) pad slot for idx==0 sessions
        T2 += 1
        R2P += 128
    spos = np.full(N, -1, np.int64)
    spos[s_sorted] = np.arange(NS)

    cs2 = np.minimum(np.arange(T2) * (128 * NCORES), NS - 1)
    k2 = (np.maximum(1, deg[s_sorted[cs2]]) + 1).astype(np.int64)  # +1 self

    groups2 = _greedy_groups(k2, 0, T2)
    g_of_tile2 = np.empty(T2, np.int64)
    moff2 = np.empty(len(groups2), np.int64)
    off = 0
    for gi, (t0, Tg, kg) in enumerate(groups2):
        g_of_tile2[t0 : t0 + Tg] = gi
        moff2[gi] = off
        off += 128 * kg * Tg
    S2G = off

    pk2 = np.zeros((NCORES, S2G), np.int32)
    pv2 = np.zeros((NCORES, S2G), np.float16)
    # self slot j=0: + h1[i]/3
    q = np.arange(NS)
    qc = q % NCORES
    qs = q // NCORES
    qt = qs // 128
    qp = qs % 128
    g_q = g_of_tile2[qt]
    t0_q = np.array([g[0] for g in groups2])[g_q]
    Tg_q = np.array([g[1] for g in groups2])[g_q]
    kg_q = np.array([g[2] for g in groups2])[g_q]
    mq = moff2[g_q] + qp * (kg_q * Tg_q) + 0 * Tg_q + (qt - t0_q)
    pk2[qc, mq] = tab_of[s_sorted].astype(np.int32)
    pv2[qc, mq] = np.float16(1.0 / 3.0)
    # edges with row in S, slots j>=1
    maskE = spos[rows] >= 0
    q_e2 = spos[rows[maskE]]
    ordE2 = np.argsort(q_e2, kind="stable")
    qe_s = q_e2[ordE2]
    j2_s = np.arange(len(qe_s)) - np.searchsorted(qe_s, qe_s, side="left")
    j2 = np.empty(len(qe_s), np.int64)
    j2[ordE2] = j2_s
    c2 = q_e2 % NCORES
    s2 = q_e2 // NCORES
    t2_ = s2 // 128
    p2 = s2 % 128
    g_2 = g_of_tile2[t2_]
    t0_2 = np.array([g[0] for g in groups2])[g_2]
    Tg_2 = np.array([g[1] for g in groups2])[g_2]
    kg_2 = np.array([g[2] for g in groups2])[g_2]
    mpos2 = moff2[g_2] + p2 * (kg_2 * Tg_2) + (1 + j2) * Tg_2 + (t2_ - t0_2)
    pk2[c2, mpos2] = col_tab[maskE]
    pv2[c2, mpos2] = (vals[maskE] / 3.0).astype(np.float16)

    # host-gathered emb/3 rows, pre-swizzled per group: flat [p][t][e]
    h0rows = np.zeros((NCORES, R2P, E), np.float16)
    h0rows[qc, qs] = (emb[s_sorted] / 3.0).astype(np.float16)
    h0sw = np.zeros((NCORES, T2 * 128 * E), np.float16)
    hoff = {}
    off = 0
    for gi, (t0, Tg, kg) in enumerate(groups2):
        hoff[gi] = off
        blk = h0rows[:, t0 * 128 : (t0 + Tg) * 128, :]  # [NC, Tg*128, E]
        blk = blk.reshape(NCORES, Tg, 128, E).transpose(0, 2, 1, 3)  # [NC,p,t,e]
        h0sw[:, off : off + 128 * Tg * E] = blk.reshape(NCORES, -1)
        off += 128 * Tg * E
    h0sw = h0sw[:, :off]
    H0SZ = off

    # ---------------- session routing ----------------
    pad_crow = 0 * R2P + used0  # all-zero row in compact table (core0 pad)
    crow = np.full((B, L), pad_crow, np.int64)
    nz = sess > 0
    qv = spos[sess[nz] - 1]
    assert (qv >= 0).all()
    crow[nz] = (qv % NCORES) * R2P + qv // NCORES
    SESS = B_LOC * L
    SJ = -(-SESS // 128)
    assert SJ * 128 == SESS, "SESS must be a multiple of 128"
    sidx = np.full((NCORES, 128, SJ), pad_crow, np.int32)
    for c in range(NCORES):
        flat = crow[c * B_LOC : (c + 1) * B_LOC].ravel()  # r = b_loc*L + l
        rr = np.arange(SESS)
        sidx[c, rr % 128, rr // 128] = flat

    mask_c = mask.reshape(NCORES, 1, SESS).astype(np.float32)
    slen_c = slen.reshape(NCORES, 1, B_LOC).astype(np.float32)

    cfg = dict(
        N=N, B=B, L=L, B_LOC=B_LOC, RLOC=RLOC, RPAD=RPAD, T1=T1,
        T2=T2, R2P=R2P, S1G=S1G, S2G=S2G, H0SZ=H0SZ, SJ=SJ, SESS=SESS,
        NCH=nch, CHT=CHT,
        groups1=groups1, groups2=groups2,
        moff1=[int(x) for x in moff1], moff2=[int(x) for x in moff2],
        hoff=hoff,
    )
    per_core = []
    for c in range(NCORES):
        per_core.append(
            dict(
                emb_pos=emb_pos,
                pk1=pk1[c], pv1=pv1[c],
                pk2=pk2[c], pv2=pv2[c],
                h0sw=h0sw[c],
                sidx=sidx[c],
                pos50=np.ascontiguousarray(posemb[:L]),
                w1t=np.ascontiguousarray(np.asarray(inputs["w_1"])[:E].astype(np.float32)),
                w1b=np.ascontiguousarray(np.asarray(inputs["w_1"])[E:].astype(np.float32)),
                g1w=np.asarray(inputs["glu1_w"]).astype(np.float32),
                g1b=np.asarray(inputs["glu1_b"]).astype(np.float32).reshape(E, 1),
                g2w=np.asarray(inputs["glu2_w"]).astype(np.float32),
                w2=np.asarray(inputs["w_2"]).astype(np.float32),
                mask_c=mask_c[c],
                slen_c=slen_c[c],
            )
        )
    return cfg, per_core


# --------------------------------------------------------------------------
# device program
# --------------------------------------------------------------------------
def _build_program(cfg, stage="full", debug_taps=False):
    import concourse.bass as bass
    import concourse.bacc as bacc
    import concourse.mybir as mybir
    import concourse.tile as tile
    from concourse.masks import make_identity

    dt = mybir.dt
    f32, f16, i32 = dt.float32, dt.float16, dt.int32
    Alu = mybir.AluOpType
    Act = mybir.ActivationFunctionType
    X = mybir.AxisListType.X

    L = cfg["L"]
    B_LOC = cfg["B_LOC"]
    RPAD = cfg["RPAD"]
    T2 = cfg["T2"]
    R2P = cfg["R2P"]
    S1G = cfg["S1G"]
    S2G = cfg["S2G"]
    H0SZ = cfg["H0SZ"]
    SJ = cfg["SJ"]
    SESS = cfg["SESS"]
    groups1 = cfg["groups1"]
    groups2 = cfg["groups2"]
    moff1 = cfg["moff1"]
    moff2 = cfg["moff2"]
    hoff = cfg["hoff"]
    NCH = cfg["NCH"]
    CHT = cfg["CHT"]
    CH = CHT * 128
    rg = [list(range(NCORES))]

    nc = bacc.Bacc(
        "TRN2", target_bir_lowering=False, debug=False, num_devices=NCORES
    )
    emb_pos = nc.dram_tensor("emb_pos", [NCORES * RPAD, E], f16, kind="ExternalInput").ap()
    pk1_d = nc.dram_tensor("pk1", [S1G], i32, kind="ExternalInput").ap()
    pv1_d = nc.dram_tensor("pv1", [S1G], f16, kind="ExternalInput").ap()
    pk2_d = nc.dram_tensor("pk2", [S2G], i32, kind="ExternalInput").ap()
    pv2_d = nc.dram_tensor("pv2", [S2G], f16, kind="ExternalInput").ap()
    h0sw_d = nc.dram_tensor("h0sw", [H0SZ], f16, kind="ExternalInput").ap()
    sidx_d = nc.dram_tensor("sidx", [128, SJ], i32, kind="ExternalInput").ap()
    pos_d = nc.dram_tensor("pos50", [L, E], f32, kind="ExternalInput").ap()
    w1t_d = nc.dram_tensor("w1t", [E, E], f32, kind="ExternalInput").ap()
    w1b_d = nc.dram_tensor("w1b", [E, E], f32, kind="ExternalInput").ap()
    g1w_d = nc.dram_tensor("g1w", [E, E], f32, kind="ExternalInput").ap()
    g1b_d = nc.dram_tensor("g1b", [E, 1], f32, kind="ExternalInput").ap()
    g2w_d = nc.dram_tensor("g2w", [E, E], f32, kind="ExternalInput").ap()
    w2_d = nc.dram_tensor("w2", [E, 1], f32, kind="ExternalInput").ap()
    mask_d = nc.dram_tensor("mask_c", [1, SESS], f32, kind="ExternalInput").ap()
    slen_d = nc.dram_tensor("slen_c", [1, B_LOC], f32, kind="ExternalInput").ap()
    out_d = nc.dram_tensor("out", [B_LOC, E], f32, kind="ExternalOutput").ap()

    if debug_taps:
        g1_0 = cfg["groups1"][0]
        dbg_g = nc.dram_tensor(
            "dbg_g", [128, g1_0[1] * g1_0[2] * E], f16, kind="ExternalOutput"
        ).ap()
        dbg_h1 = nc.dram_tensor("dbg_h1", [RPAD, E], f16, kind="ExternalOutput").ap()
        dbg_cmp = nc.dram_tensor("dbg_cmp", [R2P, E], f16, kind="ExternalOutput").ap()
    h1_blk = nc.dram_tensor("h1_blk", [RPAD, E], f16, kind="Internal").ap()
    h1_full = nc.dram_tensor(
        "h1_full", [NCORES * RPAD, E], f16, kind="Internal", addr_space="Shared"
    ).ap()
    cmp_blk = nc.dram_tensor("cmp_blk", [R2P, E], f16, kind="Internal").ap()
    cmp_full = nc.dram_tensor(
        "cmp_full", [NCORES * R2P, E], f16, kind="Internal", addr_space="Shared"
    ).ap()

    from contextlib import ExitStack

    with tile.TileContext(nc) as tc, ExitStack() as ctx:
        res = ctx.enter_context(tc.tile_pool(name="res", bufs=1))
        mpool = ctx.enter_context(tc.tile_pool(name="meta", bufs=2))
        gpool = ctx.enter_context(tc.tile_pool(name="g", bufs=2))
        hpool = ctx.enter_context(tc.tile_pool(name="h0", bufs=2))

        def sparse_layer(groups, moff, pk_d, pv_d, src_tab, dst_blk, lname,
                         with_h0=False, post_group=None):
            pending = []  # delayed actions (AG triggers), flushed mid-gather
            for gi, (t0, Tg, kg) in enumerate(groups):
                Kg = kg * Tg
                TE = Tg * E
                mi = mpool.tile([128, Kg], i32, tag=f"{lname}i", name=f"{lname}i_{gi}")
                mv = mpool.tile([128, Kg], f16, tag=f"{lname}v", name=f"{lname}v_{gi}")
                nc.sync.dma_start(
                    out=mi[:],
                    in_=pk_d[moff[gi] : moff[gi] + 128 * Kg].rearrange(
                        "(p c) -> p c", c=Kg
                    ),
                )
                nc.sync.dma_start(
                    out=mv[:],
                    in_=pv_d[moff[gi] : moff[gi] + 128 * Kg].rearrange(
                        "(p c) -> p c", c=Kg
                    ),
                )
                g = gpool.tile([128, Kg * E], f16, tag="g")
                # one indirect DMA per slot column (the only offset/run pairing
                # the SWDGE ucode supports: one offset per partition)
                flush_at = Kg - 1
                for c in range(Kg):
                    if c == flush_at:
                        for fn in pending:
                            fn()
                        pending = []
                    nc.gpsimd.indirect_dma_start(
                        out=g[:, c * E : (c + 1) * E],
                        out_offset=None,
                        in_=src_tab,
                        in_offset=bass.IndirectOffsetOnAxis(
                            ap=mi[:, c : c + 1], axis=0
                        ),
                    )
                if debug_taps and lname == "m1" and gi == 0:
                    nc.sync.dma_start(out=dbg_g, in_=g[:])
                # multiply every gathered row by its edge value (k-major
                # layout: column block c = j*Tg + t, value per (j, t))
                g4 = g[:].rearrange("p (k t e) -> p k t e", t=Tg, e=E)
                mv4 = (
                    mv[:]
                    .rearrange("p (k t) -> p k t", t=Tg)
                    .unsqueeze(3)
                    .broadcast_to([128, kg, Tg, E])
                )
                nc.vector.scalar_tensor_tensor(
                    out=g4, in0=g4, scalar=1.0, in1=mv4,
                    op0=Alu.bypass, op1=Alu.mult,
                )
                # pairwise-add tree over k (in place, contiguous halves)
                k = kg
                while k > 1:
                    half = k // 2
                    lo = g[:, : half * TE]
                    hi = g[:, (k - half) * TE : k * TE]
                    nc.vector.scalar_tensor_tensor(
                        out=lo, in0=lo, scalar=1.0, in1=hi,
                        op0=Alu.bypass, op1=Alu.add,
                    )
                    k -= half
                if with_h0:
                    h0t = hpool.tile([128, TE], f16, tag="h0t")
                    nc.sync.dma_start(
                        out=h0t[:],
                        in_=h0sw_d[hoff[gi] : hoff[gi] + 128 * TE].rearrange(
                            "(p x) -> p x", x=TE
                        ),
                    )
                    nc.vector.scalar_tensor_tensor(
                        out=g[:, :TE], in0=g[:, :TE], scalar=1.0, in1=h0t[:],
                        op0=Alu.bypass, op1=Alu.add,
                    )
                nc.sync.dma_start(
                    out=dst_blk[t0 * 128 : (t0 + Tg) * 128, :].rearrange(
                        "(t p) e -> p t e", p=128
                    ),
                    in_=g[:, :TE].rearrange("p (t e) -> p t e", e=E),
                )
                if debug_taps:
                    dbg_t = dbg_h1 if lname == "m1" else dbg_cmp
                    nc.sync.dma_start(
                        out=dbg_t[t0 * 128 : (t0 + Tg) * 128, :].rearrange(
                            "(t p) e -> p t e", p=128
                        ),
                        in_=g[:, :TE].rearrange("p (t e) -> p t e", e=E),
                    )
                if post_group is not None:
                    post_group(gi, t0, Tg, pending)
            for fn in pending:
                fn()

        def _dummy_out():
            dummy = res.tile([B_LOC, E], f32, tag="dummy", name="dummy")
            nc.vector.memset(dummy[:], 0.0)
            nc.sync.dma_start(out=out_d, in_=dummy[:])

        # ---------------- layer 1 + chunked AllGather ----------------
        def l1_post(gi, t0, Tg, pending):
            if stage == "l1":
                return
            tend = t0 + Tg
            if tend % CHT == 0:
                ch = tend // CHT - 1

                def fire(ch=ch):
                    nc.gpsimd.collective_compute(
                        "AllGather",
                        Alu.bypass,
                        replica_groups=rg,
                        ins=[h1_blk[ch * CH : (ch + 1) * CH, :]],
                        outs=[h1_full[ch * NCORES * CH : (ch + 1) * NCORES * CH, :]],
                    )

                pending.append(fire)

        sparse_layer(groups1, moff1, pk1_d, pv1_d, emb_pos, h1_blk, "m1",
                     post_group=l1_post)
        done = stage == "l1"
        if done:
            _dummy_out()

        # ---------------- layer 2 (+ emb/3) + AllGather ----------------
        if not done:
            sparse_layer(groups2, moff2, pk2_d, pv2_d, h1_full, cmp_blk, "m2",
                         with_h0=True)
            nc.gpsimd.collective_compute(
                "AllGather",
                Alu.bypass,
                replica_groups=rg,
                ins=[cmp_blk[:]],
                outs=[cmp_full[:]],
            )
            if stage == "l2":
                _dummy_out()
                done = True

        if not done:
            # ---------------- stage 2: session attention ----------------
            ident = res.tile([128, 128], f32, tag="ident")
            make_identity(nc, ident[:])

            # weights
            w1t_t = res.tile([E, E], f32, tag="w1t")
            w1b_t = res.tile([E, E], f32, tag="w1b")
            g1w_t = res.tile([E, E], f32, tag="g1w")
            g1b_t = res.tile([E, 1], f32, tag="g1b")
            g2w_t = res.tile([E, E], f32, tag="g2w")
            w2_t = res.tile([E, 1], f32, tag="w2")
            pos_t = res.tile([L, E], f32, tag="pos")
            mask_t = res.tile([1, SESS], f32, tag="maskt")
            slen_t = res.tile([1, B_LOC], f32, tag="slent")
            for tt, dd in [
                (w1t_t, w1t_d), (w1b_t, w1b_d), (g1w_t, g1w_d), (g1b_t, g1b_d),
                (g2w_t, g2w_d), (w2_t, w2_d), (pos_t, pos_d), (mask_t, mask_d),
                (slen_t, slen_d),
            ]:
                nc.sync.dma_start(out=tt[:], in_=dd)

            sidx_t = res.tile([128, SJ], i32, tag="sidxt")
            nc.sync.dma_start(out=sidx_t[:], in_=sidx_d)
            g16 = res.tile([128, SJ * E], f16, tag="g16")
            for j in range(SJ):
                nc.gpsimd.indirect_dma_start(
                    out=g16[:, j * E : (j + 1) * E],
                    out_offset=None,
                    in_=cmp_full,
                    in_offset=bass.IndirectOffsetOnAxis(
                        ap=sidx_t[:, j : j + 1], axis=0
                    ),
                )
            g32 = res.tile([128, SJ * E], f32, tag="g32")
            nc.vector.tensor_copy(out=g32[:], in_=g16[:])

            seq_T = res.tile([128, SJ * 128], f32, tag="seqT")
            nc.vector.memset(seq_T[:], 0.0)
            nh_T = res.tile([E, SESS], f32, tag="nhT")
            nh2_T = res.tile([E, SESS], f32, tag="nh2T")
            beta_t = res.tile([1, SESS], f32, tag="betat")
            wsum = res.tile([128, SESS], f32, tag="wsum")
            hs_T = res.tile([128, B_LOC], f32, tag="hsT")
            pos_rep = res.tile([E, 10 * L], f32, tag="posrep")
            ones_t = res.tile([1, 128], f32, tag="ones")
            nc.vector.memset(ones_t[:], 1.0)

            with tc.tile_pool(name="psA", bufs=2, space="PSUM") as psA, \
                 tc.tile_pool(name="psB", bufs=2, space="PSUM") as psB, \
                 tc.tile_pool(name="psC", bufs=1, space="PSUM") as psC, \
                 tc.tile_pool(name="psD", bufs=1, space="PSUM") as psD, \
                 tc.tile_pool(name="psT", bufs=2, space="PSUM") as psT:
                # transposes: seq chunks, pos_T, glu1_wT, glu2_wT
                for j in range(SJ):
                    pt = psT.tile([128, 128], f32, tag="pt")
                    nc.tensor.transpose(
                        out=pt[:E, :], in_=g32[:, j * E : (j + 1) * E],
                        identity=ident[:],
                    )
                    nc.vector.tensor_copy(
                        out=seq_T[:E, j * 128 : (j + 1) * 128], in_=pt[:E, :]
                    )
                posT_t = res.tile([E, L], f32, tag="posT")
                pt = psT.tile([128, 128], f32, tag="pt")
                nc.tensor.transpose(out=pt[:E, :L], in_=pos_t[:], identity=ident[:L, :L])
                nc.vector.tensor_copy(out=posT_t[:], in_=pt[:E, :L])
                g1wT_t = res.tile([E, E], f32, tag="g1wT")
                pt = psT.tile([128, 128], f32, tag="pt")
                nc.tensor.transpose(out=pt[:E, :E], in_=g1w_t[:], identity=ident[:E, :E])
                nc.vector.tensor_copy(out=g1wT_t[:], in_=pt[:E, :E])
                g2wT_t = res.tile([E, E], f32, tag="g2wT")
                pt = psT.tile([128, 128], f32, tag="pt")
                nc.tensor.transpose(out=pt[:E, :E], in_=g2w_t[:], identity=ident[:E, :E])
                nc.vector.tensor_copy(out=g2wT_t[:], in_=pt[:E, :E])

                # pos_rep: pos_T columns repeated for 10 sessions
                nc.vector.tensor_copy(
                    out=pos_rep[:].rearrange("p (b l) -> p b l", l=L),
                    in_=posT_t[:].unsqueeze(1).broadcast_to([E, 10, L]),
                )

                # hs_T = (sum_l seq) / len
                hsum = res.tile([128, B_LOC], f32, tag="hsum")
                nc.vector.tensor_reduce(
                    out=hsum[:],
                    in_=seq_T[:, :SESS].rearrange("p (b l) -> p b l", l=L),
                    axis=X,
                    op=Alu.add,
                )
                rcp = res.tile([1, B_LOC], f32, tag="rcp")
                nc.vector.reciprocal(out=rcp[:], in_=slen_t[:])
                pr = psT.tile([128, B_LOC], f32, tag="pt")
                nc.tensor.matmul(out=pr[:], lhsT=ones_t[:], rhs=rcp[:], start=True, stop=True)
                nc.vector.tensor_tensor(out=hs_T[:], in0=hsum[:], in1=pr[:], op=Alu.mult)

                # session chunks of 10 sessions (500 cols)
                nb = 10
                for b0 in range(0, B_LOC, nb):
                    bn = min(nb, B_LOC - b0)
                    ch = bn * L
                    c0 = b0 * L
                    pA = psA.tile([E, nb * L], f32, tag="pA")
                    nc.tensor.matmul(
                        out=pA[:, :ch], lhsT=w1b_t[:], rhs=seq_T[:E, c0 : c0 + ch],
                        start=True, stop=False,
                    )
                    nc.tensor.matmul(
                        out=pA[:, :ch], lhsT=w1t_t[:], rhs=pos_rep[:, :ch],
                        start=False, stop=True,
                    )
                    nc.scalar.activation(out=nh_T[:, c0 : c0 + ch], in_=pA[:, :ch], func=Act.Tanh)

                    hs_rep = res.tile([E, nb * L], f32, tag="hsrep")
                    nc.vector.tensor_copy(
                        out=hs_rep[:, :ch].rearrange("p (b l) -> p b l", l=L),
                        in_=hs_T[:E, b0 : b0 + bn].unsqueeze(2).broadcast_to([E, bn, L]),
                    )
                    pB = psB.tile([E, nb * L], f32, tag="pB")
                    nc.tensor.matmul(
                        out=pB[:, :ch], lhsT=g1wT_t[:], rhs=nh_T[:, c0 : c0 + ch],
                        start=True, stop=False,
                    )
                    nc.tensor.matmul(
                        out=pB[:, :ch], lhsT=g2wT_t[:], rhs=hs_rep[:, :ch],
                        start=False, stop=True,
                    )
                    nc.scalar.activation(
                        out=nh2_T[:, c0 : c0 + ch], in_=pB[:, :ch], func=Act.Sigmoid,
                        bias=g1b_t[:],
                    )
                    pC = psC.tile([1, nb * L], f32, tag="pC")
                    nc.tensor.matmul(
                        out=pC[:, :ch], lhsT=w2_t[:], rhs=nh2_T[:, c0 : c0 + ch],
                        start=True, stop=True,
                    )
                    nc.vector.tensor_tensor(
                        out=beta_t[:, c0 : c0 + ch], in0=pC[:, :ch],
                        in1=mask_t[:, c0 : c0 + ch], op=Alu.mult,
                    )
                    pD = psD.tile([128, nb * L], f32, tag="pD")
                    nc.tensor.matmul(
                        out=pD[:, :ch], lhsT=ones_t[:], rhs=beta_t[:, c0 : c0 + ch],
                        start=True, stop=True,
                    )
                    nc.vector.tensor_tensor(
                        out=wsum[:, c0 : c0 + ch], in0=seq_T[:, c0 : c0 + ch],
                        in1=pD[:, :ch], op=Alu.mult,
                    )

                sel_T = res.tile([128, B_LOC], f32, tag="selT")
                nc.vector.tensor_reduce(
                    out=sel_T[:],
                    in_=wsum[:].rearrange("p (b l) -> p b l", l=L),
                    axis=X,
                    op=Alu.add,
                )
                po = psT.tile([128, 128], f32, tag="pt")
                nc.tensor.transpose(
                    out=po[:B_LOC, :], in_=sel_T[:], identity=ident[:]
                )
                outsb = res.tile([B_LOC, 128], f32, tag="outsb")
                nc.vector.tensor_copy(out=outsb[:], in_=po[:B_LOC, :])
                nc.sync.dma_start(out=out_d, in_=outsb[:, :E])

    nc.compile()
    return nc


# --------------------------------------------------------------------------
# entry point
# --------------------------------------------------------------------------
def kernel(**inputs):
    from concourse import bass_utils

    cfg, per_core = _preprocess(inputs)
    nc = _build_program(cfg)
    in_maps = [dict(pc) for pc in per_core]
    res = bass_utils.run_bass_kernel_spmd(
        nc, in_maps, core_ids=list(range(NCORES)), trace=False
    )
    out = np.concatenate([res.results[c]["out"] for c in range(NCORES)], axis=0)
    return out.astype(np.float32)


if __name__ == "__main__":
    pass


# revision 22
# speedup vs baseline: 1.1635x; 1.0955x over previous
"""COTREC GNN message-passing kernel for 8 Trainium2 NeuronCores.

Strategy (sharding_hint: row-shard sparse mm + all-gather; data-parallel
sessions):
  - Nodes are degree-sorted and striped across the 8 cores so every core gets
    an identical per-row-tile degree profile (one SPMD program).
  - Layer 1 (h1 = A @ emb): per core, for each 128-row tile, one indirect DMA
    gathers all neighbor rows (512B padded) into slot-major SBUF layout;
    DVE multiplies by edge values and does a strided segmented reduce.
    h1 row blocks are AllGathered (chunked, overlapped with compute).
  - Layer 2 (h2 = A @ h1) is computed only for nodes actually referenced by
    sessions (~22% of nodes); (emb + h1 + h2)/3 rows are packed into a
    compact table and AllGathered.
  - Stage 2 (session attention pooling) is data-parallel over sessions (64 per
    core) in feature-major layout: PE matmuls + ACT tanh/sigmoid + DVE.
Host side does only index preprocessing / sharding / packing (numpy).
"""
import sys

sys.path.insert(0, "/opt/trn_rl_repo")

import numpy as np

NCORES = 8
EP = 128  # padded row width (fp32 elems) -> 512B rows


# --------------------------------------------------------------------------
# host preprocessing
# --------------------------------------------------------------------------
def _preprocess(inputs):
    rows = np.asarray(inputs["adj_rows"]).astype(np.int64).ravel()
    cols = np.asarray(inputs["adj_cols"]).astype(np.int64).ravel()
    vals = np.asarray(inputs["adj_vals"]).astype(np.float32).ravel()
    emb = np.asarray(inputs["embedding"]).astype(np.float32)
    sess = np.asarray(inputs["reversed_sess_item"]).astype(np.int64)
    mask = np.asarray(inputs["mask"]).astype(np.float32)
    slen = np.asarray(inputs["session_len"]).astype(np.float32)
    posemb = np.asarray(inputs["pos_embedding"]).astype(np.float32)

    N, E = emb.shape
    B, L = sess.shape
    assert B % NCORES == 0
    B_LOC = B // NCORES
    RLOC = -(-N // NCORES)
    T1 = -(-RLOC // 128)
    RPAD = T1 * 128
    if RLOC == RPAD:  # need a guaranteed-zero pad row in core0's block
        T1 += 1
        RPAD += 128

    # AllGather chunking: outputs must be contiguous, so h1_full is laid out
    # chunk-major: [chunk][rank][rows-in-chunk]
    nch = next(d for d in (7, 8, 6, 5, 4, 3, 2, 1) if T1 % d == 0)
    CHT = T1 // nch  # tiles per chunk
    CH = CHT * 128  # rows per chunk per core

    deg = np.bincount(rows, minlength=N).astype(np.int64)
    order = np.argsort(-deg, kind="stable")
    pos_of = np.empty(N, np.int64)
    pos_of[order] = np.arange(N)
    deg_sorted = deg[order]
    # table position of node n in the chunk-major AG layout
    _c = pos_of % NCORES
    _s = pos_of // NCORES
    tab_of = (_s // CH) * (NCORES * CH) + _c * CH + (_s % CH)

    # k schedule layer 1: max degree within each 128-row tile stripe
    chunk_starts = np.minimum(np.arange(T1) * (128 * NCORES), N - 1)
    k1 = np.maximum(1, deg_sorted[chunk_starts]).astype(np.int64)
    slot_off1 = np.concatenate([[0], np.cumsum(128 * k1)])[:-1]
    S1 = int((128 * k1).sum())

    # edge -> (core, tile, partition, j)
    p_e = pos_of[rows]
    c_e = (p_e % NCORES).astype(np.int64)
    loc_e = p_e // NCORES
    ordE = np.argsort(p_e, kind="stable")
    pe_s = p_e[ordE]
    j_s = np.arange(len(rows)) - np.searchsorted(pe_s, pe_s, side="left")
    j_e = np.empty(len(rows), np.int64)
    j_e[ordE] = j_s
    t_e = loc_e // 128
    part_e = loc_e % 128
    # meta layout per tile: [part][j], flat; separate idx (int32) / val (f32)
    mpos = slot_off1[t_e] + part_e * k1[t_e] + j_e
    pk1 = np.zeros((NCORES, S1), np.int32)
    pv1 = np.zeros((NCORES, S1), np.float32)
    col_tab = tab_of[cols].astype(np.int32)
    pk1[c_e, mpos] = col_tab
    pv1[c_e, mpos] = vals

    # positioned padded embedding table (zero tails, zero pad rows)
    emb_pos = np.zeros((NCORES * RPAD, EP), np.float32)
    emb_pos[tab_of[np.arange(N)], :E] = emb
    # guaranteed zero row: core0's first pad slot (local slot RLOC; RLOC < RPAD
    # is ensured above), mapped into the chunk-major layout
    zero_pos = int((RLOC // CH) * (NCORES * CH) + 0 * CH + (RLOC % CH))

    # ---------------- layer 2 (needed nodes only) ----------------
    s_nodes = np.unique(sess[sess > 0]).astype(np.int64) - 1
    s_sorted = s_nodes[np.argsort(-deg[s_nodes], kind="stable")]
    NS = len(s_sorted)
    R2 = -(-NS // NCORES)
    T2 = -(-R2 // 128)
    R2P = T2 * 128
    used0 = -(-NS // NCORES)  # occupied local slots on core 0
    if used0 >= R2P:  # need a free pad slot for idx==0 sessions
        T2 += 1
        R2P += 128
    spos = np.full(N, -1, np.int64)
    spos[s_sorted] = np.arange(NS)

    cs2 = np.minimum(np.arange(T2) * (128 * NCORES), NS - 1)
    k2 = (np.maximum(1, deg[s_sorted[cs2]]) + 1).astype(np.int64)  # +1 self slot
    slot_off2 = np.concatenate([[0], np.cumsum(128 * k2)])[:-1]
    S2 = int((128 * k2).sum())

    pk2 = np.full((NCORES, S2), zero_pos, np.int32)
    pv2 = np.zeros((NCORES, S2), np.float32)
    # self slot j=0: + h1[i]/3
    q = np.arange(NS)
    qc = q % NCORES
    qs = q // NCORES
    qt = qs // 128
    qp = qs % 128
    mq = slot_off2[qt] + qp * k2[qt]
    pk2[qc, mq] = tab_of[s_sorted].astype(np.int32)
    pv2[qc, mq] = np.float32(1.0 / 3.0)
    # edges with row in S, slots j>=1
    maskE = spos[rows] >= 0
    q_e2 = spos[rows[maskE]]
    ordE2 = np.argsort(q_e2, kind="stable")
    qe_s = q_e2[ordE2]
    j2_s = np.arange(len(qe_s)) - np.searchsorted(qe_s, qe_s, side="left")
    j2 = np.empty(len(qe_s), np.int64)
    j2[ordE2] = j2_s
    c2 = q_e2 % NCORES
    s2 = q_e2 // NCORES
    t2_ = s2 // 128
    p2 = s2 % 128
    mpos2 = slot_off2[t2_] + p2 * k2[t2_] + 1 + j2
    pk2[c2, mpos2] = col_tab[maskE]
    pv2[c2, mpos2] = vals[maskE] / 3.0

    # host-gathered h0 rows (input packing) per core
    h0s = np.zeros((NCORES, R2P, EP), np.float32)
    h0s[qc, qs, :E] = emb[s_sorted]

    # ---------------- session routing ----------------
    pad_crow = 0 * R2P + used0  # zero row in compact table (core0 pad slot)
    crow = np.full((B, L), pad_crow, np.int64)
    nz = sess > 0
    qv = spos[sess[nz] - 1]
    assert (qv >= 0).all()
    crow[nz] = (qv % NCORES) * R2P + qv // NCORES
    SESS = B_LOC * L
    SJ = -(-SESS // 128)
    SESSP = SJ * 128
    sidx = np.full((NCORES, 128, SJ), pad_crow, np.int32)
    for c in range(NCORES):
        flat = crow[c * B_LOC : (c + 1) * B_LOC].ravel()  # r = b_loc*L + l
        rr = np.arange(SESS)
        sidx[c, rr % 128, rr // 128] = flat

    mask_c = mask.reshape(NCORES, 1, SESS).astype(np.float32)
    slen_c = slen.reshape(NCORES, 1, B_LOC).astype(np.float32)

    cfg = dict(
        N=N, E=E, B=B, L=L, B_LOC=B_LOC, RLOC=RLOC, RPAD=RPAD, T1=T1,
        T2=T2, R2P=R2P, S1=S1, S2=S2, SJ=SJ, SESS=SESS, NCH=nch, CHT=CHT,
        k1=[int(x) for x in k1], k2=[int(x) for x in k2],
        off1=[int(x) for x in slot_off1], off2=[int(x) for x in slot_off2],
    )
    per_core = []
    for c in range(NCORES):
        per_core.append(
            dict(
                emb_pos=emb_pos,
                pk1=pk1[c], pv1=pv1[c],
                pk2=pk2[c], pv2=pv2[c],
                h0s=h0s[c],
                sidx=sidx[c],
                pos50=np.ascontiguousarray(posemb[:L]),
                w1t=np.ascontiguousarray(np.asarray(inputs["w_1"])[:E].astype(np.float32)),
                w1b=np.ascontiguousarray(np.asarray(inputs["w_1"])[E:].astype(np.float32)),
                g1w=np.asarray(inputs["glu1_w"]).astype(np.float32),
                g1b=np.asarray(inputs["glu1_b"]).astype(np.float32).reshape(E, 1),
                g2w=np.asarray(inputs["glu2_w"]).astype(np.float32),
                w2=np.asarray(inputs["w_2"]).astype(np.float32),
                mask_c=mask_c[c],
                slen_c=slen_c[c],
            )
        )
    return cfg, per_core


# --------------------------------------------------------------------------
# device program
# --------------------------------------------------------------------------
def _build_program(cfg, stage="full"):
    import concourse.bass as bass
    import concourse.bacc as bacc
    import concourse.mybir as mybir
    import concourse.tile as tile
    from concourse.masks import make_identity

    dt = mybir.dt
    f32, i32 = dt.float32, dt.int32
    Alu = mybir.AluOpType
    Act = mybir.ActivationFunctionType
    X = mybir.AxisListType.X

    E = cfg["E"]
    L = cfg["L"]
    B_LOC = cfg["B_LOC"]
    RPAD = cfg["RPAD"]
    T1 = cfg["T1"]
    T2 = cfg["T2"]
    R2P = cfg["R2P"]
    S1 = cfg["S1"]
    S2 = cfg["S2"]
    SJ = cfg["SJ"]
    SESS = cfg["SESS"]
    k1 = cfg["k1"]
    k2 = cfg["k2"]
    off1 = cfg["off1"]
    off2 = cfg["off2"]
    rg = [list(range(NCORES))]

    nc = bacc.Bacc(
        "TRN2", target_bir_lowering=False, debug=False, num_devices=NCORES
    )
    emb_pos = nc.dram_tensor("emb_pos", [NCORES * RPAD, EP], f32, kind="ExternalInput").ap()
    pk1_d = nc.dram_tensor("pk1", [S1], i32, kind="ExternalInput").ap()
    pv1_d = nc.dram_tensor("pv1", [S1], f32, kind="ExternalInput").ap()
    pk2_d = nc.dram_tensor("pk2", [S2], i32, kind="ExternalInput").ap()
    pv2_d = nc.dram_tensor("pv2", [S2], f32, kind="ExternalInput").ap()
    h0s_d = nc.dram_tensor("h0s", [R2P, EP], f32, kind="ExternalInput").ap()
    sidx_d = nc.dram_tensor("sidx", [128, SJ], i32, kind="ExternalInput").ap()
    pos_d = nc.dram_tensor("pos50", [L, E], f32, kind="ExternalInput").ap()
    w1t_d = nc.dram_tensor("w1t", [E, E], f32, kind="ExternalInput").ap()
    w1b_d = nc.dram_tensor("w1b", [E, E], f32, kind="ExternalInput").ap()
    g1w_d = nc.dram_tensor("g1w", [E, E], f32, kind="ExternalInput").ap()
    g1b_d = nc.dram_tensor("g1b", [E, 1], f32, kind="ExternalInput").ap()
    g2w_d = nc.dram_tensor("g2w", [E, E], f32, kind="ExternalInput").ap()
    w2_d = nc.dram_tensor("w2", [E, 1], f32, kind="ExternalInput").ap()
    mask_d = nc.dram_tensor("mask_c", [1, SESS], f32, kind="ExternalInput").ap()
    slen_d = nc.dram_tensor("slen_c", [1, B_LOC], f32, kind="ExternalInput").ap()
    out_d = nc.dram_tensor("out", [B_LOC, E], f32, kind="ExternalOutput").ap()
    dbg_rows = 2048
    dbg_d = nc.dram_tensor("dbg", [dbg_rows, EP], f32, kind="ExternalOutput").ap()
    dbg2_d = nc.dram_tensor("dbg2", [512, EP], f32, kind="ExternalOutput").ap()

    h1_blk = nc.dram_tensor("h1_blk", [RPAD, EP], f32, kind="Internal").ap()
    h1_full = nc.dram_tensor(
        "h1_full", [NCORES * RPAD, EP], f32, kind="Internal", addr_space="Shared"
    ).ap()
    cmp_blk = nc.dram_tensor("cmp_blk", [R2P, EP], f32, kind="Internal").ap()
    cmp_full = nc.dram_tensor(
        "cmp_full", [NCORES * R2P, EP], f32, kind="Internal", addr_space="Shared"
    ).ap()

    NCH = cfg["NCH"]
    CHT = cfg["CHT"]
    CH = CHT * 128  # rows per chunk per core

    from contextlib import ExitStack

    with tile.TileContext(nc) as tc, ExitStack() as ctx:
        res = ctx.enter_context(tc.tile_pool(name="res", bufs=1))
        mpool = ctx.enter_context(tc.tile_pool(name="meta", bufs=1))
        gpool = ctx.enter_context(tc.tile_pool(name="g", bufs=3))
        hpool = ctx.enter_context(tc.tile_pool(name="h0", bufs=2))

        o_t = [
            res.tile([128, EP], f32, tag=f"o{i}", name=f"o{i}") for i in range(2)
        ]
        for i in range(2):
            nc.vector.memset(o_t[i][:], 0.0)

        def layer(pk_d, pv_d, src_tab, dst_blk, T, ks, offs, lname, extra=None, post=None):
            for t in range(T):
                k = ks[t]
                mi = mpool.tile([128, k], i32, tag=f"{lname}i_{t}", name=f"{lname}i_{t}")
                mv = mpool.tile([128, k], f32, tag=f"{lname}v_{t}", name=f"{lname}v_{t}")
                nc.sync.dma_start(
                    out=mi[:],
                    in_=pk_d[offs[t] : offs[t] + 128 * k].rearrange("(p c) -> p c", c=k),
                )
                nc.sync.dma_start(
                    out=mv[:],
                    in_=pv_d[offs[t] : offs[t] + 128 * k].rearrange("(p c) -> p c", c=k),
                )
                g = gpool.tile([128, k * EP], f32, tag="g")
                for j in range(k):
                    nc.gpsimd.indirect_dma_start(
                        out=g[:, j * EP : (j + 1) * EP],
                        out_offset=None,
                        in_=src_tab,
                        in_offset=bass.IndirectOffsetOnAxis(
                            ap=mi[:, j : j + 1], axis=0
                        ),
                    )
                o = o_t[t % 2]
                tmp = gpool.tile([128, EP], f32, tag="tmp")
                for j in range(k):
                    dst = o if j == 0 else tmp
                    nc.vector.tensor_scalar(
                        out=dst[:], in0=g[:, j * EP : (j + 1) * EP],
                        scalar1=mv[:, j : j + 1], scalar2=None,
                        op0=Alu.mult,
                    )
                    if j > 0:
                        nc.vector.tensor_tensor(
                            out=o[:], in0=o[:], in1=tmp[:], op=Alu.add
                        )
                if extra is not None:
                    extra(t, o)
                nc.sync.dma_start(
                    out=dst_blk[t * 128 : (t + 1) * 128, :], in_=o[:]
                )
                if post is not None:
                    post(t)

        # ---------------- layer 1 + chunked AllGather ----------------
        # h1_full is chunk-major: [chunk][rank][CH rows]; each chunk's AG
        # output is a contiguous region (BIR requires contiguous collective
        # outputs).
        def l1_post(t):
            if stage in ("l1", "l1s"):
                return
            if (t + 1) % CHT == 0:
                g = (t + 1) // CHT - 1
                nc.gpsimd.collective_compute(
                    "AllGather",
                    Alu.bypass,
                    replica_groups=rg,
                    ins=[h1_blk[g * CH : (g + 1) * CH, :]],
                    outs=[h1_full[g * NCORES * CH : (g + 1) * NCORES * CH, :]],
                )

        def _dummy_out():
            dummy = res.tile([B_LOC, E], f32, tag="dummy", name="dummy")
            nc.vector.memset(dummy[:], 0.0)
            nc.sync.dma_start(out=out_d, in_=dummy[:])

        layer(pk1_d, pv1_d, emb_pos, h1_blk, T1, k1, off1, "m1", post=l1_post)
        done = stage in ("l1", "l1s", "l1ag")
        if done:
            _dummy_out()

        # ---------------- layer 2 (+ emb/3) + AllGather ----------------
        def l2_extra(t, o):
            h0t = hpool.tile([128, EP], f32, tag="h0t")
            nc.sync.dma_start(out=h0t[:], in_=h0s_d[t * 128 : (t + 1) * 128, :])
            nc.vector.tensor_scalar_mul(out=h0t[:], in0=h0t[:], scalar1=1.0 / 3.0)
            nc.vector.tensor_tensor(out=o[:], in0=o[:], in1=h0t[:], op=Alu.add)

        if not done:
            layer(pk2_d, pv2_d, h1_full, cmp_blk, T2, k2, off2, "m2", extra=l2_extra)
            nc.gpsimd.collective_compute(
                "AllGather",
                Alu.bypass,
                replica_groups=rg,
                ins=[cmp_blk[:]],
                outs=[cmp_full[:]],
            )
            if stage == "l2":
                _dummy_out()
                done = True

        if not done:
            # ---------------- stage 2: session attention ----------------
            ident = res.tile([128, 128], f32, tag="ident")
            make_identity(nc, ident[:])

            # weights
            w1t_t = res.tile([E, E], f32, tag="w1t")
            w1b_t = res.tile([E, E], f32, tag="w1b")
            g1w_t = res.tile([E, E], f32, tag="g1w")
            g1b_t = res.tile([E, 1], f32, tag="g1b")
            g2w_t = res.tile([E, E], f32, tag="g2w")
            w2_t = res.tile([E, 1], f32, tag="w2")
            pos_t = res.tile([L, E], f32, tag="pos")
            mask_t = res.tile([1, SESS], f32, tag="maskt")
            slen_t = res.tile([1, B_LOC], f32, tag="slent")
            for tt, dd in [
                (w1t_t, w1t_d), (w1b_t, w1b_d), (g1w_t, g1w_d), (g1b_t, g1b_d),
                (g2w_t, g2w_d), (w2_t, w2_d), (pos_t, pos_d), (mask_t, mask_d),
                (slen_t, slen_d),
            ]:
                nc.sync.dma_start(out=tt[:], in_=dd)

            sidx_t = res.tile([128, SJ], i32, tag="sidxt")
            nc.sync.dma_start(out=sidx_t[:], in_=sidx_d)
            g_sess = res.tile([128, SJ * EP], f32, tag="gsess")
            for j in range(SJ):
                nc.gpsimd.indirect_dma_start(
                    out=g_sess[:, j * EP : (j + 1) * EP],
                    out_offset=None,
                    in_=cmp_full,
                    in_offset=bass.IndirectOffsetOnAxis(
                        ap=sidx_t[:, j : j + 1], axis=0
                    ),
                )

            seq_T = res.tile([128, SJ * 128], f32, tag="seqT")
            nh_T = res.tile([E, SESS], f32, tag="nhT")
            nh2_T = res.tile([E, SESS], f32, tag="nh2T")
            beta_t = res.tile([1, SESS], f32, tag="betat")
            wsum = res.tile([128, SESS], f32, tag="wsum")
            hs_T = res.tile([128, B_LOC], f32, tag="hsT")
            pos_rep = res.tile([E, 10 * L], f32, tag="posrep")
            ones_t = res.tile([1, 128], f32, tag="ones")
            nc.vector.memset(ones_t[:], 1.0)

            with tc.tile_pool(name="psA", bufs=2, space="PSUM") as psA, \
                 tc.tile_pool(name="psB", bufs=2, space="PSUM") as psB, \
                 tc.tile_pool(name="psC", bufs=1, space="PSUM") as psC, \
                 tc.tile_pool(name="psD", bufs=1, space="PSUM") as psD, \
                 tc.tile_pool(name="psT", bufs=2, space="PSUM") as psT:
                # transposes: seq chunks, pos_T, glu1_wT, glu2_wT
                for j in range(SJ):
                    pt = psT.tile([128, 128], f32, tag="pt")
                    nc.tensor.transpose(
                        out=pt[:], in_=g_sess[:, j * EP : j * EP + 128], identity=ident[:]
                    )
                    nc.vector.tensor_copy(
                        out=seq_T[:, j * 128 : (j + 1) * 128], in_=pt[:]
                    )
                posT_t = res.tile([E, L], f32, tag="posT")
                pt = psT.tile([128, 128], f32, tag="pt")
                nc.tensor.transpose(out=pt[:E, :L], in_=pos_t[:], identity=ident[:L, :L])
                nc.vector.tensor_copy(out=posT_t[:], in_=pt[:E, :L])
                g1wT_t = res.tile([E, E], f32, tag="g1wT")
                pt = psT.tile([128, 128], f32, tag="pt")
                nc.tensor.transpose(out=pt[:E, :E], in_=g1w_t[:], identity=ident[:E, :E])
                nc.vector.tensor_copy(out=g1wT_t[:], in_=pt[:E, :E])
                g2wT_t = res.tile([E, E], f32, tag="g2wT")
                pt = psT.tile([128, 128], f32, tag="pt")
                nc.tensor.transpose(out=pt[:E, :E], in_=g2w_t[:], identity=ident[:E, :E])
                nc.vector.tensor_copy(out=g2wT_t[:], in_=pt[:E, :E])

                # pos_rep: pos_T columns repeated for 10 sessions
                nc.vector.tensor_copy(
                    out=pos_rep[:].rearrange("p (b l) -> p b l", l=L),
                    in_=posT_t[:].unsqueeze(1).broadcast_to([E, 10, L]),
                )

                # hs_T = (sum_l seq) / len
                hsum = res.tile([128, B_LOC], f32, tag="hsum")
                nc.vector.tensor_reduce(
                    out=hsum[:],
                    in_=seq_T[:, :SESS].rearrange("p (b l) -> p b l", l=L),
                    axis=X,
                    op=Alu.add,
                )
                rcp = res.tile([1, B_LOC], f32, tag="rcp")
                nc.vector.reciprocal(out=rcp[:], in_=slen_t[:])
                pr = psT.tile([128, B_LOC], f32, tag="pt")
                nc.tensor.matmul(out=pr[:], lhsT=ones_t[:], rhs=rcp[:], start=True, stop=True)
                nc.vector.tensor_tensor(out=hs_T[:], in0=hsum[:], in1=pr[:], op=Alu.mult)

                if stage == "s2a":
                    _dummy_out()

                # session chunks of 10 sessions (500 cols)
                nb = 10
                for b0 in range(0, B_LOC, nb) if stage != "s2a" else []:
                    bn = min(nb, B_LOC - b0)
                    ch = bn * L
                    c0 = b0 * L
                    pA = psA.tile([E, nb * L], f32, tag="pA")
                    nc.tensor.matmul(
                        out=pA[:, :ch], lhsT=w1b_t[:], rhs=seq_T[:E, c0 : c0 + ch],
                        start=True, stop=False,
                    )
                    nc.tensor.matmul(
                        out=pA[:, :ch], lhsT=w1t_t[:], rhs=pos_rep[:, :ch],
                        start=False, stop=True,
                    )
                    nc.scalar.activation(out=nh_T[:, c0 : c0 + ch], in_=pA[:, :ch], func=Act.Tanh)

                    hs_rep = res.tile([E, nb * L], f32, tag="hsrep")
                    nc.vector.tensor_copy(
                        out=hs_rep[:, :ch].rearrange("p (b l) -> p b l", l=L),
                        in_=hs_T[:E, b0 : b0 + bn].unsqueeze(2).broadcast_to([E, bn, L]),
                    )
                    pB = psB.tile([E, nb * L], f32, tag="pB")
                    nc.tensor.matmul(
                        out=pB[:, :ch], lhsT=g1wT_t[:], rhs=nh_T[:, c0 : c0 + ch],
                        start=True, stop=False,
                    )
                    nc.tensor.matmul(
                        out=pB[:, :ch], lhsT=g2wT_t[:], rhs=hs_rep[:, :ch],
                        start=False, stop=True,
                    )
                    nc.scalar.activation(
                        out=nh2_T[:, c0 : c0 + ch], in_=pB[:, :ch], func=Act.Sigmoid,
                        bias=g1b_t[:],
                    )
                    pC = psC.tile([1, nb * L], f32, tag="pC")
                    nc.tensor.matmul(
                        out=pC[:, :ch], lhsT=w2_t[:], rhs=nh2_T[:, c0 : c0 + ch],
                        start=True, stop=True,
                    )
                    nc.vector.tensor_tensor(
                        out=beta_t[:, c0 : c0 + ch], in0=pC[:, :ch],
                        in1=mask_t[:, c0 : c0 + ch], op=Alu.mult,
                    )
                    pD = psD.tile([128, nb * L], f32, tag="pD")
                    nc.tensor.matmul(
                        out=pD[:, :ch], lhsT=ones_t[:], rhs=beta_t[:, c0 : c0 + ch],
                        start=True, stop=True,
                    )
                    nc.vector.tensor_tensor(
                        out=wsum[:, c0 : c0 + ch], in0=seq_T[:, c0 : c0 + ch],
                        in1=pD[:, :ch], op=Alu.mult,
                    )

                if stage == "s2b":
                    _dummy_out()
                if stage not in ("s2a", "s2b"):
                    sel_T = res.tile([128, B_LOC], f32, tag="selT")
                    nc.vector.tensor_reduce(
                        out=sel_T[:],
                        in_=wsum[:].rearrange("p (b l) -> p b l", l=L),
                        axis=X,
                        op=Alu.add,
                    )
                    po = psT.tile([128, 128], f32, tag="pt")
                    nc.tensor.transpose(
                        out=po[:B_LOC, :], in_=sel_T[:], identity=ident[:]
                    )
                    outsb = res.tile([B_LOC, EP], f32, tag="outsb")
                    nc.vector.tensor_copy(out=outsb[:], in_=po[:B_LOC, :])
                    nc.sync.dma_start(out=out_d, in_=outsb[:, :E])

    nc.compile()
    return nc


# --------------------------------------------------------------------------
# entry point
# --------------------------------------------------------------------------
def kernel(**inputs):
    from concourse import bass_utils

    cfg, per_core = _preprocess(inputs)
    nc = _build_program(cfg)
    in_maps = [dict(pc) for pc in per_core]
    res = bass_utils.run_bass_kernel_spmd(
        nc, in_maps, core_ids=list(range(NCORES)), trace=False
    )
    out = np.concatenate([res.results[c]["out"] for c in range(NCORES)], axis=0)
    return out.astype(np.float32)


if __name__ == "__main__":
    pass



# revision 23
# speedup vs baseline: 1.1637x; 1.0002x over previous
"""COTREC GNN message-passing kernel for 8 Trainium2 NeuronCores.

Strategy (sharding_hint: row-shard sparse mm + all-gather; data-parallel
sessions):
  - Nodes are degree-sorted and striped across the 8 cores so every core gets
    an identical per-row-tile degree profile (one SPMD program).
  - Layer 1 (h1 = A @ emb): per core, for each 128-row tile, one indirect DMA
    gathers all neighbor rows (512B padded) into slot-major SBUF layout;
    DVE multiplies by edge values and does a strided segmented reduce.
    h1 row blocks are AllGathered (chunked, overlapped with compute).
  - Layer 2 (h2 = A @ h1) is computed only for nodes actually referenced by
    sessions (~22% of nodes); (emb + h1 + h2)/3 rows are packed into a
    compact table and AllGathered.
  - Stage 2 (session attention pooling) is data-parallel over sessions (64 per
    core) in feature-major layout: PE matmuls + ACT tanh/sigmoid + DVE.
Host side does only index preprocessing / sharding / packing (numpy).
"""
import sys

sys.path.insert(0, "/opt/trn_rl_repo")

import numpy as np

NCORES = 8
EP = 128  # padded row width (fp32 elems) -> 512B rows


# --------------------------------------------------------------------------
# host preprocessing
# --------------------------------------------------------------------------
def _preprocess(inputs):
    rows = np.asarray(inputs["adj_rows"]).astype(np.int64).ravel()
    cols = np.asarray(inputs["adj_cols"]).astype(np.int64).ravel()
    vals = np.asarray(inputs["adj_vals"]).astype(np.float32).ravel()
    emb = np.asarray(inputs["embedding"]).astype(np.float32)
    sess = np.asarray(inputs["reversed_sess_item"]).astype(np.int64)
    mask = np.asarray(inputs["mask"]).astype(np.float32)
    slen = np.asarray(inputs["session_len"]).astype(np.float32)
    posemb = np.asarray(inputs["pos_embedding"]).astype(np.float32)

    N, E = emb.shape
    B, L = sess.shape
    assert B % NCORES == 0
    B_LOC = B // NCORES
    RLOC = -(-N // NCORES)
    T1 = -(-RLOC // 128)
    RPAD = T1 * 128
    if RLOC == RPAD:  # need a guaranteed-zero pad row in core0's block
        T1 += 1
        RPAD += 128

    # AllGather chunking: outputs must be contiguous, so h1_full is laid out
    # chunk-major: [chunk][rank][rows-in-chunk]
    nch = next(d for d in (14, 16, 12, 7, 8, 6, 5, 4, 3, 2, 1) if T1 % d == 0)
    CHT = T1 // nch  # tiles per chunk
    CH = CHT * 128  # rows per chunk per core

    deg = np.bincount(rows, minlength=N).astype(np.int64)
    order = np.argsort(-deg, kind="stable")
    pos_of = np.empty(N, np.int64)
    pos_of[order] = np.arange(N)
    deg_sorted = deg[order]
    # table position of node n in the chunk-major AG layout
    _c = pos_of % NCORES
    _s = pos_of // NCORES
    tab_of = (_s // CH) * (NCORES * CH) + _c * CH + (_s % CH)

    # k schedule layer 1: max degree within each 128-row tile stripe
    chunk_starts = np.minimum(np.arange(T1) * (128 * NCORES), N - 1)
    k1 = np.maximum(1, deg_sorted[chunk_starts]).astype(np.int64)
    slot_off1 = np.concatenate([[0], np.cumsum(128 * k1)])[:-1]
    S1 = int((128 * k1).sum())

    # edge -> (core, tile, partition, j)
    p_e = pos_of[rows]
    c_e = (p_e % NCORES).astype(np.int64)
    loc_e = p_e // NCORES
    ordE = np.argsort(p_e, kind="stable")
    pe_s = p_e[ordE]
    j_s = np.arange(len(rows)) - np.searchsorted(pe_s, pe_s, side="left")
    j_e = np.empty(len(rows), np.int64)
    j_e[ordE] = j_s
    t_e = loc_e // 128
    part_e = loc_e % 128
    # meta layout per tile: [part][j], flat; separate idx (int32) / val (f32)
    mpos = slot_off1[t_e] + part_e * k1[t_e] + j_e
    pk1 = np.zeros((NCORES, S1), np.int32)
    pv1 = np.zeros((NCORES, S1), np.float32)
    col_tab = tab_of[cols].astype(np.int32)
    pk1[c_e, mpos] = col_tab
    pv1[c_e, mpos] = vals

    # positioned padded embedding table (zero tails, zero pad rows)
    emb_pos = np.zeros((NCORES * RPAD, EP), np.float32)
    emb_pos[tab_of[np.arange(N)], :E] = emb
    # guaranteed zero row: core0's first pad slot (local slot RLOC; RLOC < RPAD
    # is ensured above), mapped into the chunk-major layout
    zero_pos = int((RLOC // CH) * (NCORES * CH) + 0 * CH + (RLOC % CH))

    # ---------------- layer 2 (needed nodes only) ----------------
    s_nodes = np.unique(sess[sess > 0]).astype(np.int64) - 1
    s_sorted = s_nodes[np.argsort(-deg[s_nodes], kind="stable")]
    NS = len(s_sorted)
    R2 = -(-NS // NCORES)
    T2 = -(-R2 // 128)
    R2P = T2 * 128
    used0 = -(-NS // NCORES)  # occupied local slots on core 0
    if used0 >= R2P:  # need a free pad slot for idx==0 sessions
        T2 += 1
        R2P += 128
    spos = np.full(N, -1, np.int64)
    spos[s_sorted] = np.arange(NS)

    cs2 = np.minimum(np.arange(T2) * (128 * NCORES), NS - 1)
    k2 = (np.maximum(1, deg[s_sorted[cs2]]) + 1).astype(np.int64)  # +1 self slot
    slot_off2 = np.concatenate([[0], np.cumsum(128 * k2)])[:-1]
    S2 = int((128 * k2).sum())

    pk2 = np.full((NCORES, S2), zero_pos, np.int32)
    pv2 = np.zeros((NCORES, S2), np.float32)
    # self slot j=0: + h1[i]/3
    q = np.arange(NS)
    qc = q % NCORES
    qs = q // NCORES
    qt = qs // 128
    qp = qs % 128
    mq = slot_off2[qt] + qp * k2[qt]
    pk2[qc, mq] = tab_of[s_sorted].astype(np.int32)
    pv2[qc, mq] = np.float32(1.0 / 3.0)
    # edges with row in S, slots j>=1
    maskE = spos[rows] >= 0
    q_e2 = spos[rows[maskE]]
    ordE2 = np.argsort(q_e2, kind="stable")
    qe_s = q_e2[ordE2]
    j2_s = np.arange(len(qe_s)) - np.searchsorted(qe_s, qe_s, side="left")
    j2 = np.empty(len(qe_s), np.int64)
    j2[ordE2] = j2_s
    c2 = q_e2 % NCORES
    s2 = q_e2 // NCORES
    t2_ = s2 // 128
    p2 = s2 % 128
    mpos2 = slot_off2[t2_] + p2 * k2[t2_] + 1 + j2
    pk2[c2, mpos2] = col_tab[maskE]
    pv2[c2, mpos2] = vals[maskE] / 3.0

    # host-gathered h0 rows (input packing) per core
    h0s = np.zeros((NCORES, R2P, EP), np.float32)
    h0s[qc, qs, :E] = emb[s_sorted]

    # ---------------- session routing ----------------
    pad_crow = 0 * R2P + used0  # zero row in compact table (core0 pad slot)
    crow = np.full((B, L), pad_crow, np.int64)
    nz = sess > 0
    qv = spos[sess[nz] - 1]
    assert (qv >= 0).all()
    crow[nz] = (qv % NCORES) * R2P + qv // NCORES
    SESS = B_LOC * L
    SJ = -(-SESS // 128)
    SESSP = SJ * 128
    sidx = np.full((NCORES, 128, SJ), pad_crow, np.int32)
    for c in range(NCORES):
        flat = crow[c * B_LOC : (c + 1) * B_LOC].ravel()  # r = b_loc*L + l
        rr = np.arange(SESS)
        sidx[c, rr % 128, rr // 128] = flat

    mask_c = mask.reshape(NCORES, 1, SESS).astype(np.float32)
    slen_c = slen.reshape(NCORES, 1, B_LOC).astype(np.float32)

    cfg = dict(
        N=N, E=E, B=B, L=L, B_LOC=B_LOC, RLOC=RLOC, RPAD=RPAD, T1=T1,
        T2=T2, R2P=R2P, S1=S1, S2=S2, SJ=SJ, SESS=SESS, NCH=nch, CHT=CHT,
        k1=[int(x) for x in k1], k2=[int(x) for x in k2],
        off1=[int(x) for x in slot_off1], off2=[int(x) for x in slot_off2],
    )
    per_core = []
    for c in range(NCORES):
        per_core.append(
            dict(
                emb_pos=emb_pos,
                pk1=pk1[c], pv1=pv1[c],
                pk2=pk2[c], pv2=pv2[c],
                h0s=h0s[c],
                sidx=sidx[c],
                pos50=np.ascontiguousarray(posemb[:L]),
                w1t=np.ascontiguousarray(np.asarray(inputs["w_1"])[:E].astype(np.float32)),
                w1b=np.ascontiguousarray(np.asarray(inputs["w_1"])[E:].astype(np.float32)),
                g1w=np.asarray(inputs["glu1_w"]).astype(np.float32),
                g1b=np.asarray(inputs["glu1_b"]).astype(np.float32).reshape(E, 1),
                g2w=np.asarray(inputs["glu2_w"]).astype(np.float32),
                w2=np.asarray(inputs["w_2"]).astype(np.float32),
                mask_c=mask_c[c],
                slen_c=slen_c[c],
            )
        )
    return cfg, per_core


# --------------------------------------------------------------------------
# device program
# --------------------------------------------------------------------------
def _build_program(cfg, stage="full"):
    import concourse.bass as bass
    import concourse.bacc as bacc
    import concourse.mybir as mybir
    import concourse.tile as tile
    from concourse.masks import make_identity

    dt = mybir.dt
    f32, i32 = dt.float32, dt.int32
    Alu = mybir.AluOpType
    Act = mybir.ActivationFunctionType
    X = mybir.AxisListType.X

    E = cfg["E"]
    L = cfg["L"]
    B_LOC = cfg["B_LOC"]
    RPAD = cfg["RPAD"]
    T1 = cfg["T1"]
    T2 = cfg["T2"]
    R2P = cfg["R2P"]
    S1 = cfg["S1"]
    S2 = cfg["S2"]
    SJ = cfg["SJ"]
    SESS = cfg["SESS"]
    k1 = cfg["k1"]
    k2 = cfg["k2"]
    off1 = cfg["off1"]
    off2 = cfg["off2"]
    rg = [list(range(NCORES))]

    nc = bacc.Bacc(
        "TRN2", target_bir_lowering=False, debug=False, num_devices=NCORES
    )
    emb_pos = nc.dram_tensor("emb_pos", [NCORES * RPAD, EP], f32, kind="ExternalInput").ap()
    pk1_d = nc.dram_tensor("pk1", [S1], i32, kind="ExternalInput").ap()
    pv1_d = nc.dram_tensor("pv1", [S1], f32, kind="ExternalInput").ap()
    pk2_d = nc.dram_tensor("pk2", [S2], i32, kind="ExternalInput").ap()
    pv2_d = nc.dram_tensor("pv2", [S2], f32, kind="ExternalInput").ap()
    h0s_d = nc.dram_tensor("h0s", [R2P, EP], f32, kind="ExternalInput").ap()
    sidx_d = nc.dram_tensor("sidx", [128, SJ], i32, kind="ExternalInput").ap()
    pos_d = nc.dram_tensor("pos50", [L, E], f32, kind="ExternalInput").ap()
    w1t_d = nc.dram_tensor("w1t", [E, E], f32, kind="ExternalInput").ap()
    w1b_d = nc.dram_tensor("w1b", [E, E], f32, kind="ExternalInput").ap()
    g1w_d = nc.dram_tensor("g1w", [E, E], f32, kind="ExternalInput").ap()
    g1b_d = nc.dram_tensor("g1b", [E, 1], f32, kind="ExternalInput").ap()
    g2w_d = nc.dram_tensor("g2w", [E, E], f32, kind="ExternalInput").ap()
    w2_d = nc.dram_tensor("w2", [E, 1], f32, kind="ExternalInput").ap()
    mask_d = nc.dram_tensor("mask_c", [1, SESS], f32, kind="ExternalInput").ap()
    slen_d = nc.dram_tensor("slen_c", [1, B_LOC], f32, kind="ExternalInput").ap()
    out_d = nc.dram_tensor("out", [B_LOC, E], f32, kind="ExternalOutput").ap()
    dbg_rows = 2048
    dbg_d = nc.dram_tensor("dbg", [dbg_rows, EP], f32, kind="ExternalOutput").ap()
    dbg2_d = nc.dram_tensor("dbg2", [512, EP], f32, kind="ExternalOutput").ap()

    h1_blk = nc.dram_tensor("h1_blk", [RPAD, EP], f32, kind="Internal").ap()
    h1_full = nc.dram_tensor(
        "h1_full", [NCORES * RPAD, EP], f32, kind="Internal", addr_space="Shared"
    ).ap()
    cmp_blk = nc.dram_tensor("cmp_blk", [R2P, EP], f32, kind="Internal").ap()
    cmp_full = nc.dram_tensor(
        "cmp_full", [NCORES * R2P, EP], f32, kind="Internal", addr_space="Shared"
    ).ap()

    NCH = cfg["NCH"]
    CHT = cfg["CHT"]
    CH = CHT * 128  # rows per chunk per core

    from contextlib import ExitStack

    with tile.TileContext(nc) as tc, ExitStack() as ctx:
        res = ctx.enter_context(tc.tile_pool(name="res", bufs=1))
        mpool = ctx.enter_context(tc.tile_pool(name="meta", bufs=1))
        gpool = ctx.enter_context(tc.tile_pool(name="g", bufs=3))
        hpool = ctx.enter_context(tc.tile_pool(name="h0", bufs=2))

        o_t = [
            res.tile([128, EP], f32, tag=f"o{i}", name=f"o{i}") for i in range(2)
        ]
        for i in range(2):
            nc.vector.memset(o_t[i][:], 0.0)

        def layer(pk_d, pv_d, src_tab, dst_blk, T, ks, offs, lname, extra=None, post=None):
            for t in range(T):
                k = ks[t]
                mi = mpool.tile([128, k], i32, tag=f"{lname}i_{t}", name=f"{lname}i_{t}")
                mv = mpool.tile([128, k], f32, tag=f"{lname}v_{t}", name=f"{lname}v_{t}")
                nc.sync.dma_start(
                    out=mi[:],
                    in_=pk_d[offs[t] : offs[t] + 128 * k].rearrange("(p c) -> p c", c=k),
                )
                nc.sync.dma_start(
                    out=mv[:],
                    in_=pv_d[offs[t] : offs[t] + 128 * k].rearrange("(p c) -> p c", c=k),
                )
                g = gpool.tile([128, k * EP], f32, tag="g")
                for j in range(k):
                    nc.gpsimd.indirect_dma_start(
                        out=g[:, j * EP : (j + 1) * EP],
                        out_offset=None,
                        in_=src_tab,
                        in_offset=bass.IndirectOffsetOnAxis(
                            ap=mi[:, j : j + 1], axis=0
                        ),
                    )
                o = o_t[t % 2]
                tmp = gpool.tile([128, EP], f32, tag="tmp")
                for j in range(k):
                    dst = o if j == 0 else tmp
                    nc.vector.tensor_scalar(
                        out=dst[:], in0=g[:, j * EP : (j + 1) * EP],
                        scalar1=mv[:, j : j + 1], scalar2=None,
                        op0=Alu.mult,
                    )
                    if j > 0:
                        nc.vector.tensor_tensor(
                            out=o[:], in0=o[:], in1=tmp[:], op=Alu.add
                        )
                if extra is not None:
                    extra(t, o)
                nc.sync.dma_start(
                    out=dst_blk[t * 128 : (t + 1) * 128, :], in_=o[:]
                )
                if post is not None:
                    post(t)

        # ---------------- layer 1 + chunked AllGather ----------------
        # h1_full is chunk-major: [chunk][rank][CH rows]; each chunk's AG
        # output is a contiguous region (BIR requires contiguous collective
        # outputs).
        def l1_post(t):
            if stage in ("l1", "l1s"):
                return
            if (t + 1) % CHT == 0:
                g = (t + 1) // CHT - 1
                nc.gpsimd.collective_compute(
                    "AllGather",
                    Alu.bypass,
                    replica_groups=rg,
                    ins=[h1_blk[g * CH : (g + 1) * CH, :]],
                    outs=[h1_full[g * NCORES * CH : (g + 1) * NCORES * CH, :]],
                )

        def _dummy_out():
            dummy = res.tile([B_LOC, E], f32, tag="dummy", name="dummy")
            nc.vector.memset(dummy[:], 0.0)
            nc.sync.dma_start(out=out_d, in_=dummy[:])

        layer(pk1_d, pv1_d, emb_pos, h1_blk, T1, k1, off1, "m1", post=l1_post)
        done = stage in ("l1", "l1s", "l1ag")
        if done:
            _dummy_out()

        # ---------------- layer 2 (+ emb/3) + AllGather ----------------
        def l2_extra(t, o):
            h0t = hpool.tile([128, EP], f32, tag="h0t")
            nc.sync.dma_start(out=h0t[:], in_=h0s_d[t * 128 : (t + 1) * 128, :])
            nc.vector.tensor_scalar_mul(out=h0t[:], in0=h0t[:], scalar1=1.0 / 3.0)
            nc.vector.tensor_tensor(out=o[:], in0=o[:], in1=h0t[:], op=Alu.add)

        if not done:
            layer(pk2_d, pv2_d, h1_full, cmp_blk, T2, k2, off2, "m2", extra=l2_extra)
            nc.gpsimd.collective_compute(
                "AllGather",
                Alu.bypass,
                replica_groups=rg,
                ins=[cmp_blk[:]],
                outs=[cmp_full[:]],
            )
            if stage == "l2":
                _dummy_out()
                done = True

        if not done:
            # ---------------- stage 2: session attention ----------------
            ident = res.tile([128, 128], f32, tag="ident")
            make_identity(nc, ident[:])

            # weights
            w1t_t = res.tile([E, E], f32, tag="w1t")
            w1b_t = res.tile([E, E], f32, tag="w1b")
            g1w_t = res.tile([E, E], f32, tag="g1w")
            g1b_t = res.tile([E, 1], f32, tag="g1b")
            g2w_t = res.tile([E, E], f32, tag="g2w")
            w2_t = res.tile([E, 1], f32, tag="w2")
            pos_t = res.tile([L, E], f32, tag="pos")
            mask_t = res.tile([1, SESS], f32, tag="maskt")
            slen_t = res.tile([1, B_LOC], f32, tag="slent")
            for tt, dd in [
                (w1t_t, w1t_d), (w1b_t, w1b_d), (g1w_t, g1w_d), (g1b_t, g1b_d),
                (g2w_t, g2w_d), (w2_t, w2_d), (pos_t, pos_d), (mask_t, mask_d),
                (slen_t, slen_d),
            ]:
                nc.sync.dma_start(out=tt[:], in_=dd)

            sidx_t = res.tile([128, SJ], i32, tag="sidxt")
            nc.sync.dma_start(out=sidx_t[:], in_=sidx_d)
            g_sess = res.tile([128, SJ * EP], f32, tag="gsess")
            for j in range(SJ):
                nc.gpsimd.indirect_dma_start(
                    out=g_sess[:, j * EP : (j + 1) * EP],
                    out_offset=None,
                    in_=cmp_full,
                    in_offset=bass.IndirectOffsetOnAxis(
                        ap=sidx_t[:, j : j + 1], axis=0
                    ),
                )

            seq_T = res.tile([128, SJ * 128], f32, tag="seqT")
            nh_T = res.tile([E, SESS], f32, tag="nhT")
            nh2_T = res.tile([E, SESS], f32, tag="nh2T")
            beta_t = res.tile([1, SESS], f32, tag="betat")
            wsum = res.tile([128, SESS], f32, tag="wsum")
            hs_T = res.tile([128, B_LOC], f32, tag="hsT")
            pos_rep = res.tile([E, 10 * L], f32, tag="posrep")
            ones_t = res.tile([1, 128], f32, tag="ones")
            nc.vector.memset(ones_t[:], 1.0)

            with tc.tile_pool(name="psA", bufs=2, space="PSUM") as psA, \
                 tc.tile_pool(name="psB", bufs=2, space="PSUM") as psB, \
                 tc.tile_pool(name="psC", bufs=1, space="PSUM") as psC, \
                 tc.tile_pool(name="psD", bufs=1, space="PSUM") as psD, \
                 tc.tile_pool(name="psT", bufs=2, space="PSUM") as psT:
                # transposes: seq chunks, pos_T, glu1_wT, glu2_wT
                for j in range(SJ):
                    pt = psT.tile([128, 128], f32, tag="pt")
                    nc.tensor.transpose(
                        out=pt[:], in_=g_sess[:, j * EP : j * EP + 128], identity=ident[:]
                    )
                    nc.vector.tensor_copy(
                        out=seq_T[:, j * 128 : (j + 1) * 128], in_=pt[:]
                    )
                posT_t = res.tile([E, L], f32, tag="posT")
                pt = psT.tile([128, 128], f32, tag="pt")
                nc.tensor.transpose(out=pt[:E, :L], in_=pos_t[:], identity=ident[:L, :L])
                nc.vector.tensor_copy(out=posT_t[:], in_=pt[:E, :L])
                g1wT_t = res.tile([E, E], f32, tag="g1wT")
                pt = psT.tile([128, 128], f32, tag="pt")
                nc.tensor.transpose(out=pt[:E, :E], in_=g1w_t[:], identity=ident[:E, :E])
                nc.vector.tensor_copy(out=g1wT_t[:], in_=pt[:E, :E])
                g2wT_t = res.tile([E, E], f32, tag="g2wT")
                pt = psT.tile([128, 128], f32, tag="pt")
                nc.tensor.transpose(out=pt[:E, :E], in_=g2w_t[:], identity=ident[:E, :E])
                nc.vector.tensor_copy(out=g2wT_t[:], in_=pt[:E, :E])

                # pos_rep: pos_T columns repeated for 10 sessions
                nc.vector.tensor_copy(
                    out=pos_rep[:].rearrange("p (b l) -> p b l", l=L),
                    in_=posT_t[:].unsqueeze(1).broadcast_to([E, 10, L]),
                )

                # hs_T = (sum_l seq) / len
                hsum = res.tile([128, B_LOC], f32, tag="hsum")
                nc.vector.tensor_reduce(
                    out=hsum[:],
                    in_=seq_T[:, :SESS].rearrange("p (b l) -> p b l", l=L),
                    axis=X,
                    op=Alu.add,
                )
                rcp = res.tile([1, B_LOC], f32, tag="rcp")
                nc.vector.reciprocal(out=rcp[:], in_=slen_t[:])
                pr = psT.tile([128, B_LOC], f32, tag="pt")
                nc.tensor.matmul(out=pr[:], lhsT=ones_t[:], rhs=rcp[:], start=True, stop=True)
                nc.vector.tensor_tensor(out=hs_T[:], in0=hsum[:], in1=pr[:], op=Alu.mult)

                if stage == "s2a":
                    _dummy_out()

                # session chunks of 10 sessions (500 cols)
                nb = 10
                for b0 in range(0, B_LOC, nb) if stage != "s2a" else []:
                    bn = min(nb, B_LOC - b0)
                    ch = bn * L
                    c0 = b0 * L
                    pA = psA.tile([E, nb * L], f32, tag="pA")
                    nc.tensor.matmul(
                        out=pA[:, :ch], lhsT=w1b_t[:], rhs=seq_T[:E, c0 : c0 + ch],
                        start=True, stop=False,
                    )
                    nc.tensor.matmul(
                        out=pA[:, :ch], lhsT=w1t_t[:], rhs=pos_rep[:, :ch],
                        start=False, stop=True,
                    )
                    nc.scalar.activation(out=nh_T[:, c0 : c0 + ch], in_=pA[:, :ch], func=Act.Tanh)

                    hs_rep = res.tile([E, nb * L], f32, tag="hsrep")
                    nc.vector.tensor_copy(
                        out=hs_rep[:, :ch].rearrange("p (b l) -> p b l", l=L),
                        in_=hs_T[:E, b0 : b0 + bn].unsqueeze(2).broadcast_to([E, bn, L]),
                    )
                    pB = psB.tile([E, nb * L], f32, tag="pB")
                    nc.tensor.matmul(
                        out=pB[:, :ch], lhsT=g1wT_t[:], rhs=nh_T[:, c0 : c0 + ch],
                        start=True, stop=False,
                    )
                    nc.tensor.matmul(
                        out=pB[:, :ch], lhsT=g2wT_t[:], rhs=hs_rep[:, :ch],
                        start=False, stop=True,
                    )
                    nc.scalar.activation(
                        out=nh2_T[:, c0 : c0 + ch], in_=pB[:, :ch], func=Act.Sigmoid,
                        bias=g1b_t[:],
                    )
                    pC = psC.tile([1, nb * L], f32, tag="pC")
                    nc.tensor.matmul(
                        out=pC[:, :ch], lhsT=w2_t[:], rhs=nh2_T[:, c0 : c0 + ch],
                        start=True, stop=True,
                    )
                    nc.vector.tensor_tensor(
                        out=beta_t[:, c0 : c0 + ch], in0=pC[:, :ch],
                        in1=mask_t[:, c0 : c0 + ch], op=Alu.mult,
                    )
                    pD = psD.tile([128, nb * L], f32, tag="pD")
                    nc.tensor.matmul(
                        out=pD[:, :ch], lhsT=ones_t[:], rhs=beta_t[:, c0 : c0 + ch],
                        start=True, stop=True,
                    )
                    nc.vector.tensor_tensor(
                        out=wsum[:, c0 : c0 + ch], in0=seq_T[:, c0 : c0 + ch],
                        in1=pD[:, :ch], op=Alu.mult,
                    )

                if stage == "s2b":
                    _dummy_out()
                if stage not in ("s2a", "s2b"):
                    sel_T = res.tile([128, B_LOC], f32, tag="selT")
                    nc.vector.tensor_reduce(
                        out=sel_T[:],
                        in_=wsum[:].rearrange("p (b l) -> p b l", l=L),
                        axis=X,
                        op=Alu.add,
                    )
                    po = psT.tile([128, 128], f32, tag="pt")
                    nc.tensor.transpose(
                        out=po[:B_LOC, :], in_=sel_T[:], identity=ident[:]
                    )
                    outsb = res.tile([B_LOC, EP], f32, tag="outsb")
                    nc.vector.tensor_copy(out=outsb[:], in_=po[:B_LOC, :])
                    nc.sync.dma_start(out=out_d, in_=outsb[:, :E])

    nc.compile()
    return nc


# --------------------------------------------------------------------------
# entry point
# --------------------------------------------------------------------------
def kernel(**inputs):
    from concourse import bass_utils

    cfg, per_core = _preprocess(inputs)
    nc = _build_program(cfg)
    in_maps = [dict(pc) for pc in per_core]
    res = bass_utils.run_bass_kernel_spmd(
        nc, in_maps, core_ids=list(range(NCORES)), trace=False
    )
    out = np.concatenate([res.results[c]["out"] for c in range(NCORES)], axis=0)
    return out.astype(np.float32)


if __name__ == "__main__":
    pass



# revision 28
# speedup vs baseline: 1.1807x; 1.0146x over previous
"""COTREC GNN message-passing kernel for 8 Trainium2 NeuronCores.

Strategy (sharding_hint: row-shard sparse mm + all-gather; data-parallel
sessions):
  - Nodes are degree-sorted and striped across the 8 cores so every core gets
    an identical per-row-tile degree profile (one SPMD program).
  - Layer 1 (h1 = A @ emb): per core, for each 128-row tile, one indirect DMA
    gathers all neighbor rows (512B padded) into slot-major SBUF layout;
    DVE multiplies by edge values and does a strided segmented reduce.
    h1 row blocks are AllGathered (chunked, overlapped with compute).
  - Layer 2 (h2 = A @ h1) is computed only for nodes actually referenced by
    sessions (~22% of nodes); (emb + h1 + h2)/3 rows are packed into a
    compact table and AllGathered.
  - Stage 2 (session attention pooling) is data-parallel over sessions (64 per
    core) in feature-major layout: PE matmuls + ACT tanh/sigmoid + DVE.
Host side does only index preprocessing / sharding / packing (numpy).
"""
import sys

sys.path.insert(0, "/opt/trn_rl_repo")

import numpy as np

NCORES = 8
EP = 128  # padded row width (fp32 elems) -> 512B rows


# --------------------------------------------------------------------------
# host preprocessing
# --------------------------------------------------------------------------
def _preprocess(inputs):
    rows = np.asarray(inputs["adj_rows"]).astype(np.int64).ravel()
    cols = np.asarray(inputs["adj_cols"]).astype(np.int64).ravel()
    vals = np.asarray(inputs["adj_vals"]).astype(np.float32).ravel()
    emb = np.asarray(inputs["embedding"]).astype(np.float32)
    sess = np.asarray(inputs["reversed_sess_item"]).astype(np.int64)
    mask = np.asarray(inputs["mask"]).astype(np.float32)
    slen = np.asarray(inputs["session_len"]).astype(np.float32)
    posemb = np.asarray(inputs["pos_embedding"]).astype(np.float32)

    N, E = emb.shape
    B, L = sess.shape
    assert B % NCORES == 0
    B_LOC = B // NCORES
    RLOC = -(-N // NCORES)
    T1 = -(-RLOC // 128)
    RPAD = T1 * 128
    if RLOC == RPAD:  # need a guaranteed-zero pad row in core0's block
        T1 += 1
        RPAD += 128

    # AllGather chunking: outputs must be contiguous, so h1_full is laid out
    # chunk-major: [chunk][rank][rows-in-chunk]
    nch = next(d for d in (14, 16, 12, 7, 8, 6, 5, 4, 3, 2, 1) if T1 % d == 0)
    CHT = T1 // nch  # tiles per chunk
    CH = CHT * 128  # rows per chunk per core

    deg = np.bincount(rows, minlength=N).astype(np.int64)
    order = np.argsort(-deg, kind="stable")
    pos_of = np.empty(N, np.int64)
    pos_of[order] = np.arange(N)
    deg_sorted = deg[order]
    # table position of node n in the chunk-major AG layout
    _c = pos_of % NCORES
    _s = pos_of // NCORES
    tab_of = (_s // CH) * (NCORES * CH) + _c * CH + (_s % CH)

    # k schedule layer 1: max degree within each 128-row tile stripe
    chunk_starts = np.minimum(np.arange(T1) * (128 * NCORES), N - 1)
    k1 = np.maximum(1, deg_sorted[chunk_starts]).astype(np.int64)
    slot_off1 = np.concatenate([[0], np.cumsum(128 * k1)])[:-1]
    S1 = int((128 * k1).sum())

    # edge -> (core, tile, partition, j)
    p_e = pos_of[rows]
    c_e = (p_e % NCORES).astype(np.int64)
    loc_e = p_e // NCORES
    ordE = np.argsort(p_e, kind="stable")
    pe_s = p_e[ordE]
    j_s = np.arange(len(rows)) - np.searchsorted(pe_s, pe_s, side="left")
    j_e = np.empty(len(rows), np.int64)
    j_e[ordE] = j_s
    t_e = loc_e // 128
    part_e = loc_e % 128
    # meta layout per tile: [part][j], flat; separate idx (int32) / val (f32)
    mpos = slot_off1[t_e] + part_e * k1[t_e] + j_e
    pk1 = np.zeros((NCORES, S1), np.int32)
    pv1 = np.zeros((NCORES, S1), np.float32)
    col_tab = tab_of[cols].astype(np.int32)
    pk1[c_e, mpos] = col_tab
    pv1[c_e, mpos] = vals

    # positioned padded embedding table (zero tails, zero pad rows)
    emb_pos = np.zeros((NCORES * RPAD, EP), np.float32)
    emb_pos[tab_of[np.arange(N)], :E] = emb
    # guaranteed zero row: core0's first pad slot (local slot RLOC; RLOC < RPAD
    # is ensured above), mapped into the chunk-major layout
    zero_pos = int((RLOC // CH) * (NCORES * CH) + 0 * CH + (RLOC % CH))

    # ---------------- layer 2 (needed nodes only) ----------------
    s_nodes = np.unique(sess[sess > 0]).astype(np.int64) - 1
    s_sorted = s_nodes[np.argsort(-deg[s_nodes], kind="stable")]
    NS = len(s_sorted)
    R2 = -(-NS // NCORES)
    T2 = -(-R2 // 128)
    R2P = T2 * 128
    used0 = -(-NS // NCORES)  # occupied local slots on core 0
    if used0 >= R2P:  # need a free pad slot for idx==0 sessions
        T2 += 1
        R2P += 128
    CT2 = 3  # tiles per cmp AllGather chunk (chunk-major cmp layout)
    while T2 % CT2:
        T2 += 1
        R2P += 128
    CH2 = CT2 * 128
    spos = np.full(N, -1, np.int64)
    spos[s_sorted] = np.arange(NS)

    cs2 = np.minimum(np.arange(T2) * (128 * NCORES), NS - 1)
    k2 = (np.maximum(1, deg[s_sorted[cs2]]) + 1).astype(np.int64)  # +1 self slot
    k2[np.arange(T2) * 128 > used0] = 1  # fully-pad tiles: one column
    slot_off2 = np.concatenate([[0], np.cumsum(128 * k2)])[:-1]
    S2 = int((128 * k2).sum())

    pk2 = np.full((NCORES, S2), zero_pos, np.int32)
    pv2 = np.zeros((NCORES, S2), np.float32)
    # self slot j=0: + h1[i]/3
    q = np.arange(NS)
    qc = q % NCORES
    qs = q // NCORES
    qt = qs // 128
    qp = qs % 128
    mq = slot_off2[qt] + qp * k2[qt]
    pk2[qc, mq] = tab_of[s_sorted].astype(np.int32)
    pv2[qc, mq] = np.float32(1.0 / 3.0)
    # edges with row in S, slots j>=1
    maskE = spos[rows] >= 0
    q_e2 = spos[rows[maskE]]
    ordE2 = np.argsort(q_e2, kind="stable")
    qe_s = q_e2[ordE2]
    j2_s = np.arange(len(qe_s)) - np.searchsorted(qe_s, qe_s, side="left")
    j2 = np.empty(len(qe_s), np.int64)
    j2[ordE2] = j2_s
    c2 = q_e2 % NCORES
    s2 = q_e2 // NCORES
    t2_ = s2 // 128
    p2 = s2 % 128
    mpos2 = slot_off2[t2_] + p2 * k2[t2_] + 1 + j2
    pk2[c2, mpos2] = col_tab[maskE]
    pv2[c2, mpos2] = vals[maskE] / 3.0

    # host-gathered h0 rows (input packing) per core
    h0s = np.zeros((NCORES, R2P, EP), np.float32)
    h0s[qc, qs, :E] = emb[s_sorted]

    # ---------------- session routing (chunk-major cmp_full layout) ----------------
    def cpos(rank, slot):
        return (slot // CH2) * (NCORES * CH2) + rank * CH2 + (slot % CH2)

    pad_crow = int(cpos(0, used0))  # zero row in compact table (core0 pad slot)
    crow = np.full((B, L), pad_crow, np.int64)
    nz = sess > 0
    qv = spos[sess[nz] - 1]
    assert (qv >= 0).all()
    crow[nz] = cpos(qv % NCORES, qv // NCORES)
    SESS = B_LOC * L
    SJ = -(-SESS // 128)
    SESSP = SJ * 128
    sidx = np.full((NCORES, 128, SJ), pad_crow, np.int32)
    for c in range(NCORES):
        flat = crow[c * B_LOC : (c + 1) * B_LOC].ravel()  # r = b_loc*L + l
        rr = np.arange(SESS)
        sidx[c, rr % 128, rr // 128] = flat

    mask_c = mask.reshape(NCORES, 1, SESS).astype(np.float32)
    slen_c = slen.reshape(NCORES, 1, B_LOC).astype(np.float32)

    cfg = dict(
        N=N, E=E, B=B, L=L, B_LOC=B_LOC, RLOC=RLOC, RPAD=RPAD, T1=T1,
        T2=T2, R2P=R2P, S1=S1, S2=S2, SJ=SJ, SESS=SESS, NCH=nch, CHT=CHT,
        CT2=CT2,
        k1=[int(x) for x in k1], k2=[int(x) for x in k2],
        off1=[int(x) for x in slot_off1], off2=[int(x) for x in slot_off2],
    )
    per_core = []
    for c in range(NCORES):
        per_core.append(
            dict(
                emb_pos=emb_pos,
                pk1=pk1[c], pv1=pv1[c],
                pk2=pk2[c], pv2=pv2[c],
                h0s=h0s[c],
                sidx=sidx[c],
                pos50=np.ascontiguousarray(posemb[:L]),
                w1t=np.ascontiguousarray(np.asarray(inputs["w_1"])[:E].astype(np.float32)),
                w1b=np.ascontiguousarray(np.asarray(inputs["w_1"])[E:].astype(np.float32)),
                g1w=np.asarray(inputs["glu1_w"]).astype(np.float32),
                g1b=np.asarray(inputs["glu1_b"]).astype(np.float32).reshape(E, 1),
                g2w=np.asarray(inputs["glu2_w"]).astype(np.float32),
                w2=np.asarray(inputs["w_2"]).astype(np.float32),
                mask_c=mask_c[c],
                slen_c=slen_c[c],
            )
        )
    return cfg, per_core


# --------------------------------------------------------------------------
# device program
# --------------------------------------------------------------------------
def _build_program(cfg, stage="full"):
    import concourse.bass as bass
    import concourse.bacc as bacc
    import concourse.mybir as mybir
    import concourse.tile as tile
    from concourse.masks import make_identity

    dt = mybir.dt
    f32, i32 = dt.float32, dt.int32
    Alu = mybir.AluOpType
    Act = mybir.ActivationFunctionType
    X = mybir.AxisListType.X

    E = cfg["E"]
    L = cfg["L"]
    B_LOC = cfg["B_LOC"]
    RPAD = cfg["RPAD"]
    T1 = cfg["T1"]
    T2 = cfg["T2"]
    R2P = cfg["R2P"]
    S1 = cfg["S1"]
    S2 = cfg["S2"]
    SJ = cfg["SJ"]
    SESS = cfg["SESS"]
    k1 = cfg["k1"]
    k2 = cfg["k2"]
    off1 = cfg["off1"]
    off2 = cfg["off2"]
    rg = [list(range(NCORES))]

    nc = bacc.Bacc(
        "TRN2", target_bir_lowering=False, debug=False, num_devices=NCORES
    )
    emb_pos = nc.dram_tensor("emb_pos", [NCORES * RPAD, EP], f32, kind="ExternalInput").ap()
    pk1_d = nc.dram_tensor("pk1", [S1], i32, kind="ExternalInput").ap()
    pv1_d = nc.dram_tensor("pv1", [S1], f32, kind="ExternalInput").ap()
    pk2_d = nc.dram_tensor("pk2", [S2], i32, kind="ExternalInput").ap()
    pv2_d = nc.dram_tensor("pv2", [S2], f32, kind="ExternalInput").ap()
    h0s_d = nc.dram_tensor("h0s", [R2P, EP], f32, kind="ExternalInput").ap()
    sidx_d = nc.dram_tensor("sidx", [128, SJ], i32, kind="ExternalInput").ap()
    pos_d = nc.dram_tensor("pos50", [L, E], f32, kind="ExternalInput").ap()
    w1t_d = nc.dram_tensor("w1t", [E, E], f32, kind="ExternalInput").ap()
    w1b_d = nc.dram_tensor("w1b", [E, E], f32, kind="ExternalInput").ap()
    g1w_d = nc.dram_tensor("g1w", [E, E], f32, kind="ExternalInput").ap()
    g1b_d = nc.dram_tensor("g1b", [E, 1], f32, kind="ExternalInput").ap()
    g2w_d = nc.dram_tensor("g2w", [E, E], f32, kind="ExternalInput").ap()
    w2_d = nc.dram_tensor("w2", [E, 1], f32, kind="ExternalInput").ap()
    mask_d = nc.dram_tensor("mask_c", [1, SESS], f32, kind="ExternalInput").ap()
    slen_d = nc.dram_tensor("slen_c", [1, B_LOC], f32, kind="ExternalInput").ap()
    out_d = nc.dram_tensor("out", [B_LOC, E], f32, kind="ExternalOutput").ap()
    dbg_rows = 2048
    dbg_d = nc.dram_tensor("dbg", [dbg_rows, EP], f32, kind="ExternalOutput").ap()
    dbg2_d = nc.dram_tensor("dbg2", [512, EP], f32, kind="ExternalOutput").ap()

    h1_blk = nc.dram_tensor("h1_blk", [RPAD, EP], f32, kind="Internal").ap()
    h1_full = nc.dram_tensor(
        "h1_full", [NCORES * RPAD, EP], f32, kind="Internal", addr_space="Shared"
    ).ap()
    cmp_blk = nc.dram_tensor("cmp_blk", [R2P, EP], f32, kind="Internal").ap()
    cmp_full = nc.dram_tensor(
        "cmp_full", [NCORES * R2P, EP], f32, kind="Internal", addr_space="Shared"
    ).ap()

    NCH = cfg["NCH"]
    CHT = cfg["CHT"]
    CH = CHT * 128  # rows per chunk per core

    from contextlib import ExitStack

    with tile.TileContext(nc) as tc, ExitStack() as ctx:
        res = ctx.enter_context(tc.tile_pool(name="res", bufs=1))
        mpool = ctx.enter_context(tc.tile_pool(name="meta", bufs=1))
        gpool = ctx.enter_context(tc.tile_pool(name="g", bufs=3))
        hpool = ctx.enter_context(tc.tile_pool(name="h0", bufs=2))

        o_t = [
            res.tile([128, EP], f32, tag=f"o{i}", name=f"o{i}") for i in range(2)
        ]
        for i in range(2):
            nc.vector.memset(o_t[i][:], 0.0)

        def layer(pk_d, pv_d, src_tab, dst_blk, T, ks, offs, lname, extra=None, post=None):
            for t in range(T):
                k = ks[t]
                mi = mpool.tile([128, k], i32, tag=f"{lname}i_{t}", name=f"{lname}i_{t}")
                mv = mpool.tile([128, k], f32, tag=f"{lname}v_{t}", name=f"{lname}v_{t}")
                nc.sync.dma_start(
                    out=mi[:],
                    in_=pk_d[offs[t] : offs[t] + 128 * k].rearrange("(p c) -> p c", c=k),
                )
                nc.sync.dma_start(
                    out=mv[:],
                    in_=pv_d[offs[t] : offs[t] + 128 * k].rearrange("(p c) -> p c", c=k),
                )
                g = gpool.tile([128, k * EP], f32, tag="g")
                for j in range(k):
                    nc.gpsimd.indirect_dma_start(
                        out=g[:, j * EP : (j + 1) * EP],
                        out_offset=None,
                        in_=src_tab,
                        in_offset=bass.IndirectOffsetOnAxis(
                            ap=mi[:, j : j + 1], axis=0
                        ),
                    )
                o = o_t[t % 2]
                tmp = gpool.tile([128, EP], f32, tag="tmp")
                for j in range(k):
                    dst = o if j == 0 else tmp
                    nc.vector.tensor_scalar(
                        out=dst[:], in0=g[:, j * EP : (j + 1) * EP],
                        scalar1=mv[:, j : j + 1], scalar2=None,
                        op0=Alu.mult,
                    )
                    if j > 0:
                        nc.vector.tensor_tensor(
                            out=o[:], in0=o[:], in1=tmp[:], op=Alu.add
                        )
                if extra is not None:
                    extra(t, o)
                nc.sync.dma_start(
                    out=dst_blk[t * 128 : (t + 1) * 128, :], in_=o[:]
                )
                if post is not None:
                    post(t)

        # ---------------- layer 1 + chunked AllGather ----------------
        # h1_full is chunk-major: [chunk][rank][CH rows]; each chunk's AG
        # output is a contiguous region (BIR requires contiguous collective
        # outputs).
        def l1_post(t):
            if stage in ("l1", "l1s"):
                return
            if (t + 1) % CHT == 0:
                g = (t + 1) // CHT - 1
                nc.gpsimd.collective_compute(
                    "AllGather",
                    Alu.bypass,
                    replica_groups=rg,
                    ins=[h1_blk[g * CH : (g + 1) * CH, :]],
                    outs=[h1_full[g * NCORES * CH : (g + 1) * NCORES * CH, :]],
                )

        def _dummy_out():
            dummy = res.tile([B_LOC, E], f32, tag="dummy", name="dummy")
            nc.vector.memset(dummy[:], 0.0)
            nc.sync.dma_start(out=out_d, in_=dummy[:])

        layer(pk1_d, pv1_d, emb_pos, h1_blk, T1, k1, off1, "m1", post=l1_post)
        done = stage in ("l1", "l1s", "l1ag")
        if done:
            _dummy_out()

        # ---------------- layer 2 (+ emb/3) + AllGather ----------------
        def l2_extra(t, o):
            h0t = hpool.tile([128, EP], f32, tag="h0t")
            nc.sync.dma_start(out=h0t[:], in_=h0s_d[t * 128 : (t + 1) * 128, :])
            nc.vector.tensor_scalar_mul(out=h0t[:], in0=h0t[:], scalar1=1.0 / 3.0)
            nc.vector.tensor_tensor(out=o[:], in0=o[:], in1=h0t[:], op=Alu.add)

        CT2 = cfg["CT2"]
        CH2 = CT2 * 128

        def l2_post(t):
            if (t + 1) % CT2 == 0:
                c = (t + 1) // CT2 - 1
                nc.gpsimd.collective_compute(
                    "AllGather",
                    Alu.bypass,
                    replica_groups=rg,
                    ins=[cmp_blk[c * CH2 : (c + 1) * CH2, :]],
                    outs=[cmp_full[c * NCORES * CH2 : (c + 1) * NCORES * CH2, :]],
                )

        if not done:
            layer(pk2_d, pv2_d, h1_full, cmp_blk, T2, k2, off2, "m2",
                  extra=l2_extra, post=l2_post)
            if stage == "l2":
                _dummy_out()
                done = True

        if not done:
            # ---------------- stage 2: session attention ----------------
            ident = res.tile([128, 128], f32, tag="ident")
            make_identity(nc, ident[:])

            # weights
            w1t_t = res.tile([E, E], f32, tag="w1t")
            w1b_t = res.tile([E, E], f32, tag="w1b")
            g1w_t = res.tile([E, E], f32, tag="g1w")
            g1b_t = res.tile([E, 1], f32, tag="g1b")
            g2w_t = res.tile([E, E], f32, tag="g2w")
            w2_t = res.tile([E, 1], f32, tag="w2")
            pos_t = res.tile([L, E], f32, tag="pos")
            mask_t = res.tile([1, SESS], f32, tag="maskt")
            slen_t = res.tile([1, B_LOC], f32, tag="slent")
            for tt, dd in [
                (w1t_t, w1t_d), (w1b_t, w1b_d), (g1w_t, g1w_d), (g1b_t, g1b_d),
                (g2w_t, g2w_d), (w2_t, w2_d), (pos_t, pos_d), (mask_t, mask_d),
                (slen_t, slen_d),
            ]:
                nc.sync.dma_start(out=tt[:], in_=dd)

            sidx_t = res.tile([128, SJ], i32, tag="sidxt")
            nc.sync.dma_start(out=sidx_t[:], in_=sidx_d)
            g_sess = res.tile([128, SJ * EP], f32, tag="gsess")
            for j in range(SJ):
                nc.gpsimd.indirect_dma_start(
                    out=g_sess[:, j * EP : (j + 1) * EP],
                    out_offset=None,
                    in_=cmp_full,
                    in_offset=bass.IndirectOffsetOnAxis(
                        ap=sidx_t[:, j : j + 1], axis=0
                    ),
                )

            seq_T = res.tile([128, SJ * 128], f32, tag="seqT")
            nh_T = res.tile([E, SESS], f32, tag="nhT")
            nh2_T = res.tile([E, SESS], f32, tag="nh2T")
            beta_t = res.tile([1, SESS], f32, tag="betat")
            wsum = res.tile([128, SESS], f32, tag="wsum")
            hs_T = res.tile([128, B_LOC], f32, tag="hsT")
            pos_rep = res.tile([E, 10 * L], f32, tag="posrep")
            ones_t = res.tile([1, 128], f32, tag="ones")
            nc.vector.memset(ones_t[:], 1.0)

            with tc.tile_pool(name="psA", bufs=2, space="PSUM") as psA, \
                 tc.tile_pool(name="psB", bufs=2, space="PSUM") as psB, \
                 tc.tile_pool(name="psC", bufs=1, space="PSUM") as psC, \
                 tc.tile_pool(name="psD", bufs=1, space="PSUM") as psD, \
                 tc.tile_pool(name="psT", bufs=2, space="PSUM") as psT:
                # transposes: seq chunks, pos_T, glu1_wT, glu2_wT
                for j in range(SJ):
                    pt = psT.tile([128, 128], f32, tag="pt")
                    nc.tensor.transpose(
                        out=pt[:], in_=g_sess[:, j * EP : j * EP + 128], identity=ident[:]
                    )
                    nc.vector.tensor_copy(
                        out=seq_T[:, j * 128 : (j + 1) * 128], in_=pt[:]
                    )
                posT_t = res.tile([E, L], f32, tag="posT")
                pt = psT.tile([128, 128], f32, tag="pt")
                nc.tensor.transpose(out=pt[:E, :L], in_=pos_t[:], identity=ident[:L, :L])
                nc.vector.tensor_copy(out=posT_t[:], in_=pt[:E, :L])
                g1wT_t = res.tile([E, E], f32, tag="g1wT")
                pt = psT.tile([128, 128], f32, tag="pt")
                nc.tensor.transpose(out=pt[:E, :E], in_=g1w_t[:], identity=ident[:E, :E])
                nc.vector.tensor_copy(out=g1wT_t[:], in_=pt[:E, :E])
                g2wT_t = res.tile([E, E], f32, tag="g2wT")
                pt = psT.tile([128, 128], f32, tag="pt")
                nc.tensor.transpose(out=pt[:E, :E], in_=g2w_t[:], identity=ident[:E, :E])
                nc.vector.tensor_copy(out=g2wT_t[:], in_=pt[:E, :E])

                # pos_rep: pos_T columns repeated for 10 sessions
                nc.vector.tensor_copy(
                    out=pos_rep[:].rearrange("p (b l) -> p b l", l=L),
                    in_=posT_t[:].unsqueeze(1).broadcast_to([E, 10, L]),
                )

                # hs_T = (sum_l seq) / len
                hsum = res.tile([128, B_LOC], f32, tag="hsum")
                nc.vector.tensor_reduce(
                    out=hsum[:],
                    in_=seq_T[:, :SESS].rearrange("p (b l) -> p b l", l=L),
                    axis=X,
                    op=Alu.add,
                )
                rcp = res.tile([1, B_LOC], f32, tag="rcp")
                nc.vector.reciprocal(out=rcp[:], in_=slen_t[:])
                pr = psT.tile([128, B_LOC], f32, tag="pt")
                nc.tensor.matmul(out=pr[:], lhsT=ones_t[:], rhs=rcp[:], start=True, stop=True)
                nc.vector.tensor_tensor(out=hs_T[:], in0=hsum[:], in1=pr[:], op=Alu.mult)

                if stage == "s2a":
                    _dummy_out()

                # session chunks of 10 sessions (500 cols)
                nb = 10
                for b0 in range(0, B_LOC, nb) if stage != "s2a" else []:
                    bn = min(nb, B_LOC - b0)
                    ch = bn * L
                    c0 = b0 * L
                    pA = psA.tile([E, nb * L], f32, tag="pA")
                    nc.tensor.matmul(
                        out=pA[:, :ch], lhsT=w1b_t[:], rhs=seq_T[:E, c0 : c0 + ch],
                        start=True, stop=False,
                    )
                    nc.tensor.matmul(
                        out=pA[:, :ch], lhsT=w1t_t[:], rhs=pos_rep[:, :ch],
                        start=False, stop=True,
                    )
                    nc.scalar.activation(out=nh_T[:, c0 : c0 + ch], in_=pA[:, :ch], func=Act.Tanh)

                    hs_rep = res.tile([E, nb * L], f32, tag="hsrep")
                    nc.vector.tensor_copy(
                        out=hs_rep[:, :ch].rearrange("p (b l) -> p b l", l=L),
                        in_=hs_T[:E, b0 : b0 + bn].unsqueeze(2).broadcast_to([E, bn, L]),
                    )
                    pB = psB.tile([E, nb * L], f32, tag="pB")
                    nc.tensor.matmul(
                        out=pB[:, :ch], lhsT=g1wT_t[:], rhs=nh_T[:, c0 : c0 + ch],
                        start=True, stop=False,
                    )
                    nc.tensor.matmul(
                        out=pB[:, :ch], lhsT=g2wT_t[:], rhs=hs_rep[:, :ch],
                        start=False, stop=True,
                    )
                    nc.scalar.activation(
                        out=nh2_T[:, c0 : c0 + ch], in_=pB[:, :ch], func=Act.Sigmoid,
                        bias=g1b_t[:],
                    )
                    pC = psC.tile([1, nb * L], f32, tag="pC")
                    nc.tensor.matmul(
                        out=pC[:, :ch], lhsT=w2_t[:], rhs=nh2_T[:, c0 : c0 + ch],
                        start=True, stop=True,
                    )
                    nc.vector.tensor_tensor(
                        out=beta_t[:, c0 : c0 + ch], in0=pC[:, :ch],
                        in1=mask_t[:, c0 : c0 + ch], op=Alu.mult,
                    )
                    pD = psD.tile([128, nb * L], f32, tag="pD")
                    nc.tensor.matmul(
                        out=pD[:, :ch], lhsT=ones_t[:], rhs=beta_t[:, c0 : c0 + ch],
                        start=True, stop=True,
                    )
                    nc.vector.tensor_tensor(
                        out=wsum[:, c0 : c0 + ch], in0=seq_T[:, c0 : c0 + ch],
                        in1=pD[:, :ch], op=Alu.mult,
                    )

                if stage == "s2b":
                    _dummy_out()
                if stage not in ("s2a", "s2b"):
                    sel_T = res.tile([128, B_LOC], f32, tag="selT")
                    nc.vector.tensor_reduce(
                        out=sel_T[:],
                        in_=wsum[:].rearrange("p (b l) -> p b l", l=L),
                        axis=X,
                        op=Alu.add,
                    )
                    po = psT.tile([128, 128], f32, tag="pt")
                    nc.tensor.transpose(
                        out=po[:B_LOC, :], in_=sel_T[:], identity=ident[:]
                    )
                    outsb = res.tile([B_LOC, EP], f32, tag="outsb")
                    nc.vector.tensor_copy(out=outsb[:], in_=po[:B_LOC, :])
                    nc.sync.dma_start(out=out_d, in_=outsb[:, :E])

    nc.compile()
    return nc


# --------------------------------------------------------------------------
# entry point
# --------------------------------------------------------------------------
def kernel(**inputs):
    from concourse import bass_utils

    cfg, per_core = _preprocess(inputs)
    nc = _build_program(cfg)
    in_maps = [dict(pc) for pc in per_core]
    res = bass_utils.run_bass_kernel_spmd(
        nc, in_maps, core_ids=list(range(NCORES)), trace=False
    )
    out = np.concatenate([res.results[c]["out"] for c in range(NCORES)], axis=0)
    return out.astype(np.float32)


if __name__ == "__main__":
    pass

